# revision 24
# baseline (speedup 1.0000x reference)
"""Trainium2 Bass kernel for nn_MAB_2121713844542 (dense transformer block).

Data-parallel over batch B=32 across 8 cores (4 batches/core), activations
transposed [feature, seq] so every matmul contracts on partitions.

v2 layout/engine plan (vs baseline):
  - softmax denominators accumulate into one [4,S] PSUM tile per quad via
    masked-ones matmuls; 1/d = exp(-ln(d)) on the scalar engine (exp/ln
    tables stay resident) -- removes the 107us of DVE RECIPROCAL.
  - AV matmuls write natural head positions (tile_position col=32*h4) so
    the softmax divide + Qh residual are two full-width DVE ops per quad.
  - LayerNorm: 1/D folded into the ones-matmul weights, m2/ln/exp on ACT,
    g/beta folded into K<=1/2 broadcast matmuls, 2-op DVE tail per tile.
  - expS/Vh/G/W2 in bf16 (same PE rate, half the SBUF) which buys full
    cross-batch double buffering (bufs=2) to keep the PE fed.
"""

import functools

import numpy as np
import ml_dtypes

import concourse.bass as bass
import concourse.mybir as mybir
import concourse.tile as tile
from concourse import bacc
from concourse import hw_specs as _hw_specs
from concourse.bass_utils import run_bass_kernel_spmd

# The act-table chooser greedily picks the first table containing the needed
# function, so an Exp..Ln..Exp sequence ping-pongs between `exp_and_others`
# and `natural_log` (9 table loads per batch, ~1.5us each). Empty every table
# except the two we want so exp/ln/square/copy all resolve to
# `natural_log_exp_and_others` (ids keep their canonical positions).
_KEEP_TABLES = ("natural_log_exp_and_others", "gelu_and_others")
_orig_get_tables = _hw_specs.get_activation_tables


@functools.cache
def _patched_get_tables(arch):
    tabs = _orig_get_tables(arch)
    return {k: (v if k in _KEEP_TABLES else set()) for k, v in tabs.items()}


_hw_specs.get_activation_tables = _patched_get_tables
bacc.get_activation_tables = _patched_get_tables

B, S, D, H, DH, DFF = 32, 512, 256, 8, 32, 2048
NCORES = 8
BL = B // NCORES
P = 128
DT = D // P     # 2 feature tiles
FT = DFF // P   # 16 ffn tiles
ST = S // P     # 4 seq tiles
f32 = mybir.dt.float32
f32r = mybir.dt.float32r
bf16 = mybir.dt.bfloat16
AF = mybir.ActivationFunctionType
ALU = mybir.AluOpType
EPS = 1e-5


def build_nc():
    nc = bacc.Bacc("TRN2", target_bir_lowering=False, debug=False,
                   num_devices=NCORES)

    QT = nc.dram_tensor("QT", (BL, P, DT, S), bf16, kind="ExternalInput")
    KT = nc.dram_tensor("KT", (BL, P, DT, S), bf16, kind="ExternalInput")
    pT = nc.dram_tensor("pT", (BL, 4, S), bf16, kind="ExternalInput")
    Wq = nc.dram_tensor("Wq", (P, DT, D), bf16, kind="ExternalInput")
    Wv33 = nc.dram_tensor("Wv33", (P, DT, 264), bf16, kind="ExternalInput")
    WqA = nc.dram_tensor("WqA", (P, DT, 4, P), bf16, kind="ExternalInput")
    WkA = nc.dram_tensor("WkA", (P, DT, 4, P), bf16, kind="ExternalInput")
    WpAq = nc.dram_tensor("WpAq", (4, 4, P), bf16, kind="ExternalInput")
    WpAk = nc.dram_tensor("WpAk", (4, 4, P), bf16, kind="ExternalInput")
    W1 = nc.dram_tensor("W1", (P, DT, DFF), bf16, kind="ExternalInput")
    W2b = nc.dram_tensor("W2b", (P, FT, D), bf16, kind="ExternalInput")
    bq = nc.dram_tensor("bq", (P, DT), f32, kind="ExternalInput")
    bvb33 = nc.dram_tensor("bvb33", (1, 264), bf16, kind="ExternalInput")
    b1 = nc.dram_tensor("b1", (P, FT), f32, kind="ExternalInput")
    b2 = nc.dram_tensor("b2", (P, DT), f32, kind="ExternalInput")
    g0r = nc.dram_tensor("g0r", (1, D), f32r, kind="ExternalInput")
    nb0 = nc.dram_tensor("nb0", (1, D), f32r, kind="ExternalInput")
    g1r = nc.dram_tensor("g1r", (1, D), f32r, kind="ExternalInput")
    nb1 = nc.dram_tensor("nb1", (1, D), f32r, kind="ExternalInput")
    one33 = nc.dram_tensor("one33", (P, 2, 33), f32r, kind="ExternalInput")
    EB1 = nc.dram_tensor("EB1", (1, 4, P), f32r, kind="ExternalInput")
    onesS = nc.dram_tensor("onesS", (1, S), f32r, kind="ExternalInput")
    onesSb = nc.dram_tensor("onesSb", (1, S), bf16, kind="ExternalInput")
    outT = nc.dram_tensor("outT", (BL, P, DT, S), f32, kind="ExternalOutput")

    with tile.TileContext(nc) as tc:
        with (
            tc.tile_pool(name="singles", bufs=1) as singles,
            tc.tile_pool(name="dbl", bufs=2) as dbl,
            tc.tile_pool(name="ps_mm", bufs=3, space="PSUM") as ps_mm,
            tc.tile_pool(name="ps_acc", bufs=1, space="PSUM") as ps_acc,
            tc.tile_pool(name="ps_attn", bufs=4, space="PSUM") as ps_attn,
        ):
            def load(dram, shape):
                t = singles.tile(list(shape), dram.dtype, name="w_" + dram.name)
                nc.sync.dma_start(t, dram[tuple(slice(None) for _ in shape)])
                return t

            # order matters: proj weights first so batch 0 starts early
            Wq_sb = load(Wq, (P, DT, D))
            Wv_sb = load(Wv33, (P, DT, 264))
            bv33_sb = load(bvb33, (1, 264))
            WqA_sb = load(WqA, (P, DT, 4, P))
            WkA_sb = load(WkA, (P, DT, 4, P))
            WpAq_sb = load(WpAq, (4, 4, P))
            WpAk_sb = load(WpAk, (4, 4, P))
            EB1_sb = load(EB1, (1, 4, P))
            one33_sb = load(one33, (P, 2, 33))
            onesS_sb = load(onesS, (1, S))
            onesSb_sb = load(onesSb, (1, S))
            g0_sb = load(g0r, (1, D))
            nb0_sb = load(nb0, (1, D))
            g1_sb = load(g1r, (1, D))
            nb1_sb = load(nb1, (1, D))

            def loadj(dram, shape):
                # stage through DVE so TensorScalar-ish consumers get a
                # same-engine dep (few sync-wait slots on those structs)
                st = load(dram, shape)
                t = singles.tile(list(shape), f32, name="j_" + dram.name)
                nc.vector.tensor_copy(t, st)
                return t

            bq_sb = loadj(bq, (P, DT))
            b1_sb = loadj(b1, (P, FT))
            b2_sb = loadj(b2, (P, DT))

            W1_sb = load(W1, (P, DT, DFF))
            W2_sb = load(W2b, (P, FT, D))

            eps1 = singles.tile([1, 1], f32)
            nc.vector.memset(eps1, EPS)
            neghalf = singles.tile([1, 1], f32)
            nc.vector.memset(neghalf, -0.5)
            dummy = singles.tile([1, 1], f32)
            nc.vector.memset(dummy, 1.0)

            def layer_norm(x_sb, grow, nbrow, out_sb):
                """out = LN(x) * g + beta.  x_sb [P,DT,S] f32r."""
                x2 = dbl.tile([P, DT, S], f32r, tag="x2", bufs=1, name="x2")
                for t in range(DT):
                    nc.vector.tensor_mul(x2[:, t, :], x_sb[:, t, :],
                                         x_sb[:, t, :])
                # partition 0 <- mean, partition 32 <- E[x^2]
                acc = ps_acc.tile([33, S], f32, tag="acc", name="acc")
                for t in range(DT):
                    nc.tensor.matmul(acc, one33_sb[:, 0, :], x_sb[:, t, :],
                                     start=(t == 0), stop=False)
                for t in range(DT):
                    nc.tensor.matmul(acc, one33_sb[:, 1, :], x2[:, t, :],
                                     start=False, stop=(t == DT - 1))
                rstd = dbl.tile([1, S], f32r, tag="rstd", name="rstd")
                m2v = dbl.tile([1, S], f32r, tag="m2v", name="m2v")
                cst = dbl.tile([1, S], f32r, tag="cst", name="cst")
                nc.scalar.activation(m2v, acc[0:1, :], AF.Square)
                nc.vector.tensor_sub(m2v, acc[32:33, :], m2v)
                nc.scalar.activation(acc[32:33, :], m2v, AF.Ln, bias=eps1)
                # rstd = exp(-0.5*ln(var+eps))
                nc.scalar.activation(rstd, acc[32:33, :], AF.Exp,
                                     scale=neghalf)
                # C = mean * rstd
                nc.vector.tensor_mul(cst, acc[0:1, :], rstd)
                layer_norm.rstd = rstd
                for t in range(DT):
                    bcA = ps_attn.tile([P, S], f32, tag="attn", name="bcA")
                    nc.tensor.matmul(bcA, grow[0:1, t * P:(t + 1) * P],
                                     rstd, start=True, stop=True)
                    bcC = ps_attn.tile([P, S], f32, tag="attn", name="bcC")
                    nc.tensor.matmul(bcC, grow[0:1, t * P:(t + 1) * P],
                                     cst, start=True, stop=False)
                    nc.tensor.matmul(bcC, nbrow[0:1, t * P:(t + 1) * P],
                                     onesS_sb, start=False, stop=True)
                    # out = x*(g*rstd) - (g*mean*rstd - beta)
                    nc.vector.tensor_mul(out_sb[:, t, :], x_sb[:, t, :], bcA)
                    nc.vector.tensor_sub(out_sb[:, t, :], out_sb[:, t, :], bcC)

            for b in range(BL):
                # ---- input loads (prefetched via bufs=2 rotation) ----
                QT_sb = dbl.tile([P, DT, S], bf16, tag="qt", name="QT_sb")
                nc.sync.dma_start(QT_sb, QT[b])
                KT_sb = dbl.tile([P, DT, S], bf16, tag="kt", name="KT_sb")
                nc.sync.dma_start(KT_sb, KT[b])
                pT_sb = dbl.tile([4, S], bf16, tag="pt", name="pT_sb")
                nc.sync.dma_start(pT_sb, pT[b])

                # ---- projections ----
                # natural Qh (for the attention residual)
                Qh = dbl.tile([P, DT, S], bf16, tag="qh", name="Qh")
                for t in range(DT):
                    ps = ps_mm.tile([P, S], f32, tag="mm", name="psq")
                    for kt in range(DT):
                        nc.tensor.matmul(
                            ps, Wq_sb[:, kt, t * P:(t + 1) * P],
                            QT_sb[:, kt, :],
                            start=(kt == 0), stop=(kt == DT - 1))
                    nc.vector.tensor_tensor(
                        Qh[:, t, :], ps,
                        bq_sb[:, t:t + 1].to_broadcast((P, S)), ALU.add)
                # aug tiles for scores: tile j partitions =
                # [Qh(2j) | Ph(2j) | Qh(2j+1) | Ph(2j+1)], biases folded via
                # the pT ones-row, so one K=64 matmul per (head, kt) yields
                # QK^T + PP^T in a single accumulation
                QA = dbl.tile([P, 4, S], bf16, tag="qa", name="QA")
                KA = dbl.tile([P, 4, S], bf16, tag="ka", name="KA")
                for j in range(4):
                    ps = ps_mm.tile([P, S], f32, tag="mm", name="psqa")
                    for kt in range(DT):
                        nc.tensor.matmul(ps, WqA_sb[:, kt, j, :],
                                         QT_sb[:, kt, :],
                                         start=(kt == 0), stop=False)
                    nc.tensor.matmul(ps, WpAq_sb[:, j, :], pT_sb,
                                     start=False, stop=True)
                    nc.vector.tensor_copy(QA[:, j, :], ps)
                    ps = ps_mm.tile([P, S], f32, tag="mm", name="pska")
                    for kt in range(DT):
                        nc.tensor.matmul(ps, WkA_sb[:, kt, j, :],
                                         KT_sb[:, kt, :],
                                         start=(kt == 0), stop=False)
                    nc.tensor.matmul(ps, WpAk_sb[:, j, :], pT_sb,
                                     start=False, stop=True)
                    nc.vector.tensor_copy(KA[:, j, :], ps)

                # V in 33-col head blocks [V_h | 1]; bias + the ones col
                # come from a K=1 matmul against the pT ones-row
                Vh = dbl.tile([P, ST, 264], bf16, tag="vh", name="Vh")
                for st in range(ST):
                    ps = ps_mm.tile([P, S], f32, tag="mm", name="psv")
                    for kt in range(DT):
                        nc.tensor.matmul(
                            ps[:, :264], KT_sb[:, kt, st * P:(st + 1) * P],
                            Wv_sb[:, kt, :],
                            start=(kt == 0), stop=False)
                    nc.tensor.matmul(
                        ps[:, :264], onesSb_sb[0:1, st * P:(st + 1) * P],
                        bv33_sb, start=False, stop=True)
                    nc.vector.tensor_copy(Vh[:, st, :], ps[:, :264])

                # ---- attention ----
                OT = dbl.tile([P, DT, S], f32r, tag="ot", name="OT")
                for quad in range(2):
                    expS = [dbl.tile([P, ST, S], bf16, tag=f"e{i}",
                                     name=f"expS{i}") for i in range(4)]
                    # paired AV: heads (0,2) in av02 rows 0-32 / 64-96
                    # (denominators at rows 32 and 96), heads (1,3) in av13
                    av02 = ps_attn.tile([P, S], f32, tag="attn", name="av02")
                    av13 = ps_attn.tile([P, S], f32, tag="attn", name="av13")
                    avt = {0: (av02, 0), 1: (av13, 0), 2: (av02, 64),
                           3: (av13, 64)}
                    sc_ps = {}
                    for kt in range(ST):
                        for h4 in range(4):
                            base = 64 * (h4 % 2)
                            j = 2 * quad + h4 // 2
                            ps = ps_mm.tile([P, S], f32, tag="mm", name="pssc")
                            sc_ps[h4] = ps
                            nc.tensor.matmul(
                                ps,
                                KA[base:base + 64, j, kt * P:(kt + 1) * P],
                                QA[base:base + 64, j, :],
                                start=True, stop=True,
                                tile_position=(base, 0))
                        for h4 in range(4):
                            nc.scalar.activation(expS[h4][:, kt, :],
                                                 sc_ps[h4], AF.Exp)
                        for h4 in range(4):
                            h = 4 * quad + h4
                            av, row = avt[h4]
                            nc.tensor.matmul(
                                av[row:row + 33, :],
                                Vh[:, kt, 33 * h:33 * h + 33],
                                expS[h4][:, kt, :],
                                start=(kt == 0), stop=(kt == ST - 1),
                                tile_position=(0, row),
                                skip_group_check=True)

                    # broadcast denominators to all 128 partitions
                    bc = ps_attn.tile([P, S], f32, tag="attn", name="bc")
                    for h4 in range(4):
                        av, row = avt[h4]
                        dn = dbl.tile([1, S], f32r, tag=f"d{h4}",
                                      name=f"dn{h4}")
                        nc.vector.tensor_copy(dn, av[row + 32:row + 33, :])
                        nc.tensor.matmul(bc, EB1_sb[0:1, h4, :], dn,
                                         start=(h4 == 0), stop=(h4 == 3),
                                         skip_group_check=True)
                    # 1/den fused into the PSUM->SBUF move (~18-bit approx)
                    bcS = dbl.tile([P, S], f32, tag="bcs", name="bcS")
                    nc.vector.reciprocal_approx_fast(bcS, bc)
                    # OT = AV/d + Qh, gathered to natural head order
                    for h4 in range(4):
                        av, row = avt[h4]
                        o0 = 32 * h4
                        nc.vector.tensor_mul(
                            OT[o0:o0 + 32, quad, :],
                            av[row:row + 32, :], bcS[o0:o0 + 32, :])
                        nc.vector.tensor_add(
                            OT[o0:o0 + 32, quad, :],
                            OT[o0:o0 + 32, quad, :],
                            Qh[o0:o0 + 32, quad, :])

                LN1 = dbl.tile([P, DT, S], bf16, tag="ln1", name="LN1")
                layer_norm(OT, g0_sb, nb0_sb, LN1)
                # prefetch the gelu table; input dep on LN1's rstd pins this
                # after LN1's Exp in the ACT queue (scheduler can't hoist it)
                nc.scalar.activation(dummy, layer_norm.rstd[0:1, 0:1], AF.Gelu)

                # ---- FFN ----
                G = dbl.tile([P, FT, S], bf16, tag="g", bufs=1, name="G")
                for ft in range(FT):
                    ps = ps_mm.tile([P, S], f32, tag="mm", name="psf")
                    for t in range(DT):
                        nc.tensor.matmul(
                            ps, W1_sb[:, t, ft * P:(ft + 1) * P],
                            LN1[:, t, :],
                            start=(t == 0), stop=(t == DT - 1))
                    nc.scalar.activation(G[:, ft, :], ps, AF.Gelu,
                                         bias=b1_sb[:, ft:ft + 1])
                # prefetch the ln/exp table; dep on the last gelu's output
                # pins it after the gelu loop in the ACT queue
                nc.scalar.activation(dummy, G[0:1, FT - 1, 0:1], AF.Ln)
                Z = dbl.tile([P, DT, S], f32r, tag="z", bufs=1, name="Z")
                for t in range(DT):
                    ps = ps_mm.tile([P, S], f32, tag="mm", name="psf2")
                    for ft in range(FT):
                        nc.tensor.matmul(
                            ps, W2_sb[:, ft, t * P:(t + 1) * P],
                            G[:, ft, :],
                            start=(ft == 0), stop=(ft == FT - 1))
                    nc.vector.tensor_add(Z[:, t, :], ps, LN1[:, t, :])
                    nc.vector.tensor_tensor(
                        Z[:, t, :], Z[:, t, :],
                        b2_sb[:, t:t + 1].to_broadcast((P, S)), ALU.add)

                OUT = dbl.tile([P, DT, S], f32, tag="out", name="OUT")
                layer_norm(Z, g1_sb, nb1_sb, OUT)
                nc.sync.dma_start(outT[b], OUT)

    nc.finalize()
    return nc


_NC = None


def kernel(Q, K, p, Wq, bq, Wk, bk, Wv, bv, Wp, bp, g0, beta0, W1, b1, W2, b2,
           g1, beta1):
    global _NC
    if _NC is None:
        _NC = build_nc()

    f = np.float32
    bf = ml_dtypes.bfloat16

    def feat_tiles(x):  # [B, S, D] -> [B, P, DT, S]
        x = np.asarray(x, f).transpose(0, 2, 1).reshape(-1, DT, P, S)
        return np.ascontiguousarray(x.transpose(0, 2, 1, 3))

    def pp(vec, n):  # [n*P] -> [P, n]
        return np.ascontiguousarray(np.asarray(vec, f).reshape(n, P).T)

    def wmat(w, n, m):  # [n*P, m] -> [P, n, m]
        w = np.asarray(w, f).reshape(n, P, m)
        return np.ascontiguousarray(w.transpose(1, 0, 2))

    QTf = feat_tiles(Q)
    KTf = feat_tiles(K)
    # p padded to 4 channels; row 3 = ones (carries the PE-proj bias).
    # PE projection pre-scaled by 1/4 so PhPh^T carries the 1/sqrt(DV)=1/16.
    pTf = np.zeros((B, 4, S), f)
    pTf[:, :3, :] = np.transpose(np.asarray(p, f), (0, 2, 1))
    pTf[:, 3, :] = 1.0
    # aug score weights: out tile j partitions =
    # [Qh(2j) | Ph(2j) | Qh(2j+1) | Ph(2j+1)]; pT row3==1 carries biases;
    # PE term pre-scaled by 1/4 each side so PhPh^T carries 1/sqrt(DV)=1/16
    Wq_f = np.asarray(Wq, f)
    Wk_f = np.asarray(Wk, f)
    Wp_f = np.asarray(Wp, f) * 0.25
    bq_f = np.asarray(bq, f)
    bk_f = np.asarray(bk, f)
    bp_f = np.asarray(bp, f) * 0.25

    def aug_w(W):  # [D, D] -> [P, DT, 4, P] lhsT tiles
        out = np.zeros((P, DT, 4, P), f)
        Wt = W.reshape(DT, P, D)  # [kt, row, out_feature]
        for j in range(4):
            for hh in range(2):
                h = 2 * j + hh
                out[:, :, j, 64 * hh:64 * hh + 32] = \
                    Wt[:, :, 32 * h:32 * h + 32].transpose(1, 0, 2)
        return out

    def aug_p(bias):  # [4, 4, P]: rows 0-2 Wp at P slots, row 3 biases
        out = np.zeros((4, 4, P), f)
        for j in range(4):
            for hh in range(2):
                h = 2 * j + hh
                out[:3, j, 64 * hh + 32:64 * hh + 64] = \
                    Wp_f[:, 32 * h:32 * h + 32]
                out[3, j, 64 * hh:64 * hh + 32] = bias[32 * h:32 * h + 32]
                out[3, j, 64 * hh + 32:64 * hh + 64] = \
                    bp_f[32 * h:32 * h + 32]
        return out

    # EB1[0, h4, :]: ones at cols 32*h4..32*h4+31 (K=1 den broadcast)
    EB1m = np.zeros((1, 4, P), f)
    for h4 in range(4):
        EB1m[0, h4, 32 * h4:32 * h4 + 32] = 1.0
    # Wv in 33-col head blocks (33h+32 left zero: ones col comes from the
    # bias matmul), bvb33 row: bv in 33-blocks with 1.0 at each 33h+32
    Wv_t = wmat(Wv, DT, D)  # [P, DT, D]
    Wv33m = np.zeros((P, DT, 264), f)
    bv33m = np.zeros((1, 264), f)
    bv_f = np.asarray(bv, f)
    for h in range(H):
        Wv33m[:, :, 33 * h:33 * h + 32] = Wv_t[:, :, 32 * h:32 * h + 32]
        bv33m[0, 33 * h:33 * h + 32] = bv_f[32 * h:32 * h + 32]
        bv33m[0, 33 * h + 32] = 1.0
    # LN partition-sum weights (1/D folded in): [:,0,:] puts sum(x)/D at
    # out partition 0, [:,1,:] puts sum(x^2)/D at out partition 32
    one33m = np.zeros((P, 2, 33), f)
    one33m[:, 0, 0] = 1.0 / D
    one33m[:, 1, 32] = 1.0 / D

    shared = {
        "Wq": wmat(Wq, DT, D).astype(bf),
        "Wv33": Wv33m.astype(bf), "bvb33": bv33m.astype(bf),
        "WqA": aug_w(Wq_f).astype(bf), "WkA": aug_w(Wk_f).astype(bf),
        "WpAq": aug_p(bq_f).astype(bf), "WpAk": aug_p(bk_f).astype(bf),
        "W1": wmat(W1, DT, DFF).astype(bf),
        "W2b": wmat(W2, FT, D).astype(bf),
        "bq": pp(bq, DT),
        "b1": pp(b1, FT), "b2": pp(b2, DT),
        "g0r": np.asarray(g0, f).reshape(1, D),
        "nb0": -np.asarray(beta0, f).reshape(1, D),
        "g1r": np.asarray(g1, f).reshape(1, D),
        "nb1": -np.asarray(beta1, f).reshape(1, D),
        "one33": one33m,
        "EB1": EB1m,
        "onesS": np.ones((1, S), f),
        "onesSb": np.ones((1, S), f).astype(bf),
    }
    in_maps = []
    for c in range(NCORES):
        m = dict(shared)
        m["QT"] = np.ascontiguousarray(QTf[c * BL:(c + 1) * BL]).astype(bf)
        m["KT"] = np.ascontiguousarray(KTf[c * BL:(c + 1) * BL]).astype(bf)
        m["pT"] = np.ascontiguousarray(pTf[c * BL:(c + 1) * BL]).astype(bf)
        in_maps.append(m)

    import os
    trace = bool(os.environ.get("BASS_TRACE"))
    res = run_bass_kernel_spmd(_NC, in_maps, core_ids=list(range(NCORES)),
                               trace=trace)
    kernel._LAST = res
    outs = [res.results[c]["outT"] for c in range(NCORES)]
    full = np.concatenate(outs, axis=0)  # [B, P, DT, S]
    full = full.transpose(0, 2, 1, 3).reshape(B, D, S)  # [B, D, S]
    return np.ascontiguousarray(full.transpose(0, 2, 1))


# revision 25
# speedup vs baseline: 1.3777x; 1.3777x over previous
"""Trainium2 Bass kernel for nn_MAB_2121713844542 (dense transformer block).

Data-parallel over batch B=32 across 8 cores (4 batches/core), activations
transposed [feature, seq] so every matmul contracts on partitions.

v2 layout/engine plan (vs baseline):
  - softmax denominators accumulate into one [4,S] PSUM tile per quad via
    masked-ones matmuls; 1/d = exp(-ln(d)) on the scalar engine (exp/ln
    tables stay resident) -- removes the 107us of DVE RECIPROCAL.
  - AV matmuls write natural head positions (tile_position col=32*h4) so
    the softmax divide + Qh residual are two full-width DVE ops per quad.
  - LayerNorm: 1/D folded into the ones-matmul weights, m2/ln/exp on ACT,
    g/beta folded into K<=1/2 broadcast matmuls, 2-op DVE tail per tile.
  - expS/Vh/G/W2 in bf16 (same PE rate, half the SBUF) which buys full
    cross-batch double buffering (bufs=2) to keep the PE fed.
"""

import functools

import numpy as np
import ml_dtypes

import concourse.bass as bass
import concourse.mybir as mybir
import concourse.tile as tile
from concourse import bacc
from concourse import hw_specs as _hw_specs
from concourse.bass_utils import run_bass_kernel_spmd

# The act-table chooser greedily picks the first table containing the needed
# function, so an Exp..Ln..Exp sequence ping-pongs between `exp_and_others`
# and `natural_log` (9 table loads per batch, ~1.5us each). Empty every table
# except the two we want so exp/ln/square/copy all resolve to
# `natural_log_exp_and_others` (ids keep their canonical positions).
_KEEP_TABLES = ("natural_log_exp_and_others", "gelu_and_others")
_orig_get_tables = _hw_specs.get_activation_tables


@functools.cache
def _patched_get_tables(arch):
    tabs = _orig_get_tables(arch)
    return {k: (v if k in _KEEP_TABLES else set()) for k, v in tabs.items()}


_hw_specs.get_activation_tables = _patched_get_tables
bacc.get_activation_tables = _patched_get_tables

B, S, D, H, DH, DFF = 32, 512, 256, 8, 32, 2048
NCORES = 8
BL = B // NCORES
P = 128
DT = D // P     # 2 feature tiles
FT = DFF // P   # 16 ffn tiles
ST = S // P     # 4 seq tiles
f32 = mybir.dt.float32
f32r = mybir.dt.float32r
bf16 = mybir.dt.bfloat16
AF = mybir.ActivationFunctionType
ALU = mybir.AluOpType
EPS = 1e-5


def build_nc():
    nc = bacc.Bacc("TRN2", target_bir_lowering=False, debug=False,
                   num_devices=NCORES)

    QT = nc.dram_tensor("QT", (BL, P, DT, S), bf16, kind="ExternalInput")
    KT = nc.dram_tensor("KT", (BL, P, DT, S), bf16, kind="ExternalInput")
    pT = nc.dram_tensor("pT", (BL, 4, S), bf16, kind="ExternalInput")
    Wq = nc.dram_tensor("Wq", (P, DT, D), bf16, kind="ExternalInput")
    Wv = nc.dram_tensor("Wv", (P, DT, D), bf16, kind="ExternalInput")
    WqA = nc.dram_tensor("WqA", (P, DT, 4, P), bf16, kind="ExternalInput")
    WkA = nc.dram_tensor("WkA", (P, DT, 4, P), bf16, kind="ExternalInput")
    WpAq = nc.dram_tensor("WpAq", (4, 4, P), bf16, kind="ExternalInput")
    WpAk = nc.dram_tensor("WpAk", (4, 4, P), bf16, kind="ExternalInput")
    W1 = nc.dram_tensor("W1", (P, DT, DFF), bf16, kind="ExternalInput")
    W2b = nc.dram_tensor("W2b", (P, FT, D), bf16, kind="ExternalInput")
    bq = nc.dram_tensor("bq", (P, DT), f32, kind="ExternalInput")
    bvb = nc.dram_tensor("bvb", (P, D), f32, kind="ExternalInput")
    b1 = nc.dram_tensor("b1", (P, FT), f32, kind="ExternalInput")
    b2 = nc.dram_tensor("b2", (P, DT), f32, kind="ExternalInput")
    g0r = nc.dram_tensor("g0r", (1, D), f32r, kind="ExternalInput")
    nb0 = nc.dram_tensor("nb0", (1, D), f32r, kind="ExternalInput")
    g1r = nc.dram_tensor("g1r", (1, D), f32r, kind="ExternalInput")
    nb1 = nc.dram_tensor("nb1", (1, D), f32r, kind="ExternalInput")
    one33 = nc.dram_tensor("one33", (P, 2, 33), f32r, kind="ExternalInput")
    Ed4 = nc.dram_tensor("Ed4", (P, 4, 4), bf16, kind="ExternalInput")
    EB4 = nc.dram_tensor("EB4", (4, P), f32r, kind="ExternalInput")
    onesS = nc.dram_tensor("onesS", (1, S), f32r, kind="ExternalInput")
    outT = nc.dram_tensor("outT", (BL, P, DT, S), f32, kind="ExternalOutput")

    with tile.TileContext(nc) as tc:
        with (
            tc.tile_pool(name="singles", bufs=1) as singles,
            tc.tile_pool(name="dbl", bufs=2) as dbl,
            tc.tile_pool(name="ps_mm", bufs=3, space="PSUM") as ps_mm,
            tc.tile_pool(name="ps_acc", bufs=1, space="PSUM") as ps_acc,
            tc.tile_pool(name="ps_av", bufs=2, space="PSUM") as ps_av,
            tc.tile_pool(name="ps_bc", bufs=2, space="PSUM") as ps_bc,
        ):
            def load(dram, shape):
                t = singles.tile(list(shape), dram.dtype, name="w_" + dram.name)
                nc.sync.dma_start(t, dram[tuple(slice(None) for _ in shape)])
                return t

            # order matters: proj weights first so batch 0 starts early
            Wq_sb = load(Wq, (P, DT, D))
            Wv_sb = load(Wv, (P, DT, D))
            WqA_sb = load(WqA, (P, DT, 4, P))
            WkA_sb = load(WkA, (P, DT, 4, P))
            WpAq_sb = load(WpAq, (4, 4, P))
            WpAk_sb = load(WpAk, (4, 4, P))
            Ed4_sb = load(Ed4, (P, 4, 4))
            EB4_sb = load(EB4, (4, P))
            one33_sb = load(one33, (P, 2, 33))
            onesS_sb = load(onesS, (1, S))
            g0_sb = load(g0r, (1, D))
            nb0_sb = load(nb0, (1, D))
            g1_sb = load(g1r, (1, D))
            nb1_sb = load(nb1, (1, D))

            def loadj(dram, shape):
                # stage through DVE so TensorScalar-ish consumers get a
                # same-engine dep (few sync-wait slots on those structs)
                st = load(dram, shape)
                t = singles.tile(list(shape), f32, name="j_" + dram.name)
                nc.vector.tensor_copy(t, st)
                return t

            bq_sb = loadj(bq, (P, DT))
            bvb_sb = loadj(bvb, (P, D))
            b1_sb = loadj(b1, (P, FT))
            b2_sb = loadj(b2, (P, DT))

            W1_sb = load(W1, (P, DT, DFF))
            W2_sb = load(W2b, (P, FT, D))

            eps1 = singles.tile([1, 1], f32)
            nc.vector.memset(eps1, EPS)
            neghalf = singles.tile([1, 1], f32)
            nc.vector.memset(neghalf, -0.5)
            dummy = singles.tile([1, 1], f32)
            nc.vector.memset(dummy, 1.0)

            def layer_norm(x_sb, grow, nbrow, out_sb):
                """out = LN(x) * g + beta.  x_sb [P,DT,S] f32r."""
                x2 = dbl.tile([P, DT, S], f32r, tag="x2", bufs=1, name="x2")
                for t in range(DT):
                    nc.vector.tensor_mul(x2[:, t, :], x_sb[:, t, :],
                                         x_sb[:, t, :])
                # partition 0 <- mean, partition 32 <- E[x^2]
                acc = ps_acc.tile([33, S], f32, tag="acc", name="acc")
                for t in range(DT):
                    nc.tensor.matmul(acc, one33_sb[:, 0, :], x_sb[:, t, :],
                                     start=(t == 0), stop=False)
                for t in range(DT):
                    nc.tensor.matmul(acc, one33_sb[:, 1, :], x2[:, t, :],
                                     start=False, stop=(t == DT - 1))
                rstd = dbl.tile([1, S], f32r, tag="rstd", name="rstd")
                m2v = dbl.tile([1, S], f32r, tag="m2v", name="m2v")
                cst = dbl.tile([1, S], f32r, tag="cst", name="cst")
                nc.scalar.activation(m2v, acc[0:1, :], AF.Square)
                nc.vector.tensor_sub(m2v, acc[32:33, :], m2v)
                nc.scalar.activation(acc[32:33, :], m2v, AF.Ln, bias=eps1)
                # rstd = exp(-0.5*ln(var+eps))
                nc.scalar.activation(rstd, acc[32:33, :], AF.Exp,
                                     scale=neghalf)
                # C = mean * rstd
                nc.vector.tensor_mul(cst, acc[0:1, :], rstd)
                layer_norm.rstd = rstd
                for t in range(DT):
                    bcA = ps_bc.tile([P, S], f32, tag="bc", name="bcA")
                    nc.tensor.matmul(bcA, grow[0:1, t * P:(t + 1) * P],
                                     rstd, start=True, stop=True)
                    bcC = ps_bc.tile([P, S], f32, tag="bc", name="bcC")
                    nc.tensor.matmul(bcC, grow[0:1, t * P:(t + 1) * P],
                                     cst, start=True, stop=False)
                    nc.tensor.matmul(bcC, nbrow[0:1, t * P:(t + 1) * P],
                                     onesS_sb, start=False, stop=True)
                    # out = x*(g*rstd) - (g*mean*rstd - beta)
                    nc.vector.tensor_mul(out_sb[:, t, :], x_sb[:, t, :], bcA)
                    nc.vector.tensor_sub(out_sb[:, t, :], out_sb[:, t, :], bcC)

            for b in range(BL):
                # ---- input loads (prefetched via bufs=2 rotation) ----
                QT_sb = dbl.tile([P, DT, S], bf16, tag="qt", name="QT_sb")
                nc.sync.dma_start(QT_sb, QT[b])
                KT_sb = dbl.tile([P, DT, S], bf16, tag="kt", name="KT_sb")
                nc.sync.dma_start(KT_sb, KT[b])
                pT_sb = dbl.tile([4, S], bf16, tag="pt", name="pT_sb")
                nc.sync.dma_start(pT_sb, pT[b])

                # ---- projections ----
                # natural Qh (for the attention residual)
                Qh = dbl.tile([P, DT, S], bf16, tag="qh", name="Qh")
                for t in range(DT):
                    ps = ps_mm.tile([P, S], f32, tag="mm", name="psq")
                    for kt in range(DT):
                        nc.tensor.matmul(
                            ps, Wq_sb[:, kt, t * P:(t + 1) * P],
                            QT_sb[:, kt, :],
                            start=(kt == 0), stop=(kt == DT - 1))
                    nc.vector.tensor_tensor(
                        Qh[:, t, :], ps,
                        bq_sb[:, t:t + 1].to_broadcast((P, S)), ALU.add)
                # aug tiles for scores: tile j partitions =
                # [Qh(2j) | Ph(2j) | Qh(2j+1) | Ph(2j+1)], biases folded via
                # the pT ones-row, so one K=64 matmul per (head, kt) yields
                # QK^T + PP^T in a single accumulation
                QA = dbl.tile([P, 4, S], bf16, tag="qa", name="QA")
                KA = dbl.tile([P, 4, S], bf16, tag="ka", name="KA")
                for j in range(4):
                    ps = ps_mm.tile([P, S], f32, tag="mm", name="psqa")
                    for kt in range(DT):
                        nc.tensor.matmul(ps, WqA_sb[:, kt, j, :],
                                         QT_sb[:, kt, :],
                                         start=(kt == 0), stop=False)
                    nc.tensor.matmul(ps, WpAq_sb[:, j, :], pT_sb,
                                     start=False, stop=True)
                    nc.vector.tensor_copy(QA[:, j, :], ps)
                    ps = ps_mm.tile([P, S], f32, tag="mm", name="pska")
                    for kt in range(DT):
                        nc.tensor.matmul(ps, WkA_sb[:, kt, j, :],
                                         KT_sb[:, kt, :],
                                         start=(kt == 0), stop=False)
                    nc.tensor.matmul(ps, WpAk_sb[:, j, :], pT_sb,
                                     start=False, stop=True)
                    nc.vector.tensor_copy(KA[:, j, :], ps)

                # V in natural layout [keys, feat], bf16, bias fused in move
                Vh = dbl.tile([P, ST, D], bf16, tag="vh", name="Vh")
                for st in range(ST):
                    ps = ps_mm.tile([P, S], f32, tag="mm", name="psv")
                    for kt in range(DT):
                        nc.tensor.matmul(
                            ps[:, :D], KT_sb[:, kt, st * P:(st + 1) * P],
                            Wv_sb[:, kt, :],
                            start=(kt == 0), stop=(kt == DT - 1))
                    nc.vector.tensor_add(Vh[:, st, :], ps[:, :D], bvb_sb)

                # ---- attention ----
                OT = dbl.tile([P, DT, S], f32r, tag="ot", name="OT")
                for quad in range(2):
                    expS = [dbl.tile([P, ST, S], bf16, tag=f"e{i}",
                                     name=f"expS{i}") for i in range(4)]
                    den = ps_acc.tile([4, S], f32, tag="acc", name="den")
                    av = ps_av.tile([P, S], f32, tag="av", name="av")
                    sc_ps = {}
                    for kt in range(ST):
                        for h4 in range(4):
                            base = 64 * (h4 % 2)
                            j = 2 * quad + h4 // 2
                            ps = ps_mm.tile([P, S], f32, tag="mm", name="pssc")
                            sc_ps[h4] = ps
                            nc.tensor.matmul(
                                ps,
                                KA[base:base + 64, j, kt * P:(kt + 1) * P],
                                QA[base:base + 64, j, :],
                                start=True, stop=True,
                                tile_position=(base, 0))
                        for h4 in range(4):
                            nc.scalar.activation(expS[h4][:, kt, :],
                                                 sc_ps[h4], AF.Exp)
                        for h4 in range(4):
                            h = 4 * quad + h4
                            nc.tensor.matmul(
                                den, Ed4_sb[:, h4, :], expS[h4][:, kt, :],
                                start=(kt == 0 and h4 == 0),
                                stop=(kt == ST - 1 and h4 == 3),
                                skip_group_check=True)
                            nc.tensor.matmul(
                                av[32 * h4:32 * h4 + 32, :],
                                Vh[:, kt, 32 * h:32 * h + 32],
                                expS[h4][:, kt, :],
                                start=(kt == 0), stop=(kt == ST - 1),
                                tile_position=(0, 32 * h4),
                                skip_group_check=True)

                    # bc = broadcast(1/den): recip fused into the move
                    r4f = dbl.tile([4, S], f32, tag="r4f", name="r4f")
                    nc.vector.reciprocal_approx_fast(r4f, den[0:4, :])
                    r4 = dbl.tile([4, S], f32r, tag="r4", name="r4")
                    nc.vector.tensor_copy(r4, r4f)
                    bc = ps_bc.tile([P, S], f32, tag="bc", name="bc")
                    nc.tensor.matmul(bc, EB4_sb, r4, start=True, stop=True)
                    bcS = dbl.tile([P, S], f32, tag="bcs", name="bcS")
                    nc.vector.tensor_copy(bcS, bc)
                    nc.vector.tensor_mul(OT[:, quad, :], av, bcS)
                    nc.vector.tensor_add(OT[:, quad, :], OT[:, quad, :],
                                         Qh[:, quad, :])

                LN1 = dbl.tile([P, DT, S], bf16, tag="ln1", name="LN1")
                layer_norm(OT, g0_sb, nb0_sb, LN1)
                # prefetch the gelu table; input dep on LN1's rstd pins this
                # after LN1's Exp in the ACT queue (scheduler can't hoist it)
                nc.scalar.activation(dummy, layer_norm.rstd[0:1, 0:1], AF.Gelu)

                # ---- FFN ----
                G = dbl.tile([P, FT, S], bf16, tag="g", bufs=1, name="G")
                for ft in range(FT):
                    ps = ps_mm.tile([P, S], f32, tag="mm", name="psf")
                    for t in range(DT):
                        nc.tensor.matmul(
                            ps, W1_sb[:, t, ft * P:(ft + 1) * P],
                            LN1[:, t, :],
                            start=(t == 0), stop=(t == DT - 1))
                    nc.scalar.activation(G[:, ft, :], ps, AF.Gelu,
                                         bias=b1_sb[:, ft:ft + 1])
                # prefetch the ln/exp table; dep on the last gelu's output
                # pins it after the gelu loop in the ACT queue
                nc.scalar.activation(dummy, G[0:1, FT - 1, 0:1], AF.Ln)
                Z = dbl.tile([P, DT, S], f32r, tag="z", bufs=1, name="Z")
                for t in range(DT):
                    ps = ps_mm.tile([P, S], f32, tag="mm", name="psf2")
                    for ft in range(FT):
                        nc.tensor.matmul(
                            ps, W2_sb[:, ft, t * P:(t + 1) * P],
                            G[:, ft, :],
                            start=(ft == 0), stop=(ft == FT - 1))
                    nc.vector.tensor_add(Z[:, t, :], ps, LN1[:, t, :])
                    nc.vector.tensor_tensor(
                        Z[:, t, :], Z[:, t, :],
                        b2_sb[:, t:t + 1].to_broadcast((P, S)), ALU.add)

                OUT = dbl.tile([P, DT, S], f32, tag="out", name="OUT")
                layer_norm(Z, g1_sb, nb1_sb, OUT)
                nc.sync.dma_start(outT[b], OUT)

    nc.finalize()
    return nc


_NC = None


def kernel(Q, K, p, Wq, bq, Wk, bk, Wv, bv, Wp, bp, g0, beta0, W1, b1, W2, b2,
           g1, beta1):
    global _NC
    if _NC is None:
        _NC = build_nc()

    f = np.float32
    bf = ml_dtypes.bfloat16

    def feat_tiles(x):  # [B, S, D] -> [B, P, DT, S]
        x = np.asarray(x, f).transpose(0, 2, 1).reshape(-1, DT, P, S)
        return np.ascontiguousarray(x.transpose(0, 2, 1, 3))

    def pp(vec, n):  # [n*P] -> [P, n]
        return np.ascontiguousarray(np.asarray(vec, f).reshape(n, P).T)

    def wmat(w, n, m):  # [n*P, m] -> [P, n, m]
        w = np.asarray(w, f).reshape(n, P, m)
        return np.ascontiguousarray(w.transpose(1, 0, 2))

    QTf = feat_tiles(Q)
    KTf = feat_tiles(K)
    # p padded to 4 channels; row 3 = ones (carries the PE-proj bias).
    # PE projection pre-scaled by 1/4 so PhPh^T carries the 1/sqrt(DV)=1/16.
    pTf = np.zeros((B, 4, S), f)
    pTf[:, :3, :] = np.transpose(np.asarray(p, f), (0, 2, 1))
    pTf[:, 3, :] = 1.0
    # aug score weights: out tile j partitions =
    # [Qh(2j) | Ph(2j) | Qh(2j+1) | Ph(2j+1)]; pT row3==1 carries biases;
    # PE term pre-scaled by 1/4 each side so PhPh^T carries 1/sqrt(DV)=1/16
    Wq_f = np.asarray(Wq, f)
    Wk_f = np.asarray(Wk, f)
    Wp_f = np.asarray(Wp, f) * 0.25
    bq_f = np.asarray(bq, f)
    bk_f = np.asarray(bk, f)
    bp_f = np.asarray(bp, f) * 0.25

    def aug_w(W):  # [D, D] -> [P, DT, 4, P] lhsT tiles
        out = np.zeros((P, DT, 4, P), f)
        Wt = W.reshape(DT, P, D)  # [kt, row, out_feature]
        for j in range(4):
            for hh in range(2):
                h = 2 * j + hh
                out[:, :, j, 64 * hh:64 * hh + 32] = \
                    Wt[:, :, 32 * h:32 * h + 32].transpose(1, 0, 2)
        return out

    def aug_p(bias):  # [4, 4, P]: rows 0-2 Wp at P slots, row 3 biases
        out = np.zeros((4, 4, P), f)
        for j in range(4):
            for hh in range(2):
                h = 2 * j + hh
                out[:3, j, 64 * hh + 32:64 * hh + 64] = \
                    Wp_f[:, 32 * h:32 * h + 32]
                out[3, j, 64 * hh:64 * hh + 32] = bias[32 * h:32 * h + 32]
                out[3, j, 64 * hh + 32:64 * hh + 64] = \
                    bp_f[32 * h:32 * h + 32]
        return out

    # EB4: r4 row h4 -> out partitions 32*h4..32*h4+31
    EB4m = np.zeros((4, P), f)
    for h4 in range(4):
        EB4m[h4, 32 * h4:32 * h4 + 32] = 1.0
    # Ed4[:, h4, :]: all-ones col h4 (masked partition-sum lhsT)
    Ed4m = np.zeros((P, 4, 4), f)
    for h4 in range(4):
        Ed4m[:, h4, h4] = 1.0
    # LN partition-sum weights (1/D folded in): [:,0,:] puts sum(x)/D at
    # out partition 0, [:,1,:] puts sum(x^2)/D at out partition 32
    one33m = np.zeros((P, 2, 33), f)
    one33m[:, 0, 0] = 1.0 / D
    one33m[:, 1, 32] = 1.0 / D

    shared = {
        "Wq": wmat(Wq, DT, D).astype(bf),
        "Wv": wmat(Wv, DT, D).astype(bf),
        "WqA": aug_w(Wq_f).astype(bf), "WkA": aug_w(Wk_f).astype(bf),
        "WpAq": aug_p(bq_f).astype(bf), "WpAk": aug_p(bk_f).astype(bf),
        "W1": wmat(W1, DT, DFF).astype(bf),
        "W2b": wmat(W2, FT, D).astype(bf),
        "bq": pp(bq, DT),
        "bvb": np.ascontiguousarray(np.broadcast_to(np.asarray(bv, f), (P, D))),
        "b1": pp(b1, FT), "b2": pp(b2, DT),
        "g0r": np.asarray(g0, f).reshape(1, D),
        "nb0": -np.asarray(beta0, f).reshape(1, D),
        "g1r": np.asarray(g1, f).reshape(1, D),
        "nb1": -np.asarray(beta1, f).reshape(1, D),
        "one33": one33m,
        "Ed4": Ed4m.astype(bf), "EB4": EB4m,
        "onesS": np.ones((1, S), f),
    }
    in_maps = []
    for c in range(NCORES):
        m = dict(shared)
        m["QT"] = np.ascontiguousarray(QTf[c * BL:(c + 1) * BL]).astype(bf)
        m["KT"] = np.ascontiguousarray(KTf[c * BL:(c + 1) * BL]).astype(bf)
        m["pT"] = np.ascontiguousarray(pTf[c * BL:(c + 1) * BL]).astype(bf)
        in_maps.append(m)

    import os
    trace = bool(os.environ.get("BASS_TRACE"))
    res = run_bass_kernel_spmd(_NC, in_maps, core_ids=list(range(NCORES)),
                               trace=trace)
    kernel._LAST = res
    outs = [res.results[c]["outT"] for c in range(NCORES)]
    full = np.concatenate(outs, axis=0)  # [B, P, DT, S]
    full = full.transpose(0, 2, 1, 3).reshape(B, D, S)  # [B, D, S]
    return np.ascontiguousarray(full.transpose(0, 2, 1))


# revision 26
# speedup vs baseline: 1.3970x; 1.0139x over previous
"""Trainium2 Bass kernel for nn_MAB_2121713844542 (dense transformer block).

Data-parallel over batch B=32 across 8 cores (4 batches/core), activations
transposed [feature, seq] so every matmul contracts on partitions.

v2 layout/engine plan (vs baseline):
  - softmax denominators accumulate into one [4,S] PSUM tile per quad via
    masked-ones matmuls; 1/d = exp(-ln(d)) on the scalar engine (exp/ln
    tables stay resident) -- removes the 107us of DVE RECIPROCAL.
  - AV matmuls write natural head positions (tile_position col=32*h4) so
    the softmax divide + Qh residual are two full-width DVE ops per quad.
  - LayerNorm: 1/D folded into the ones-matmul weights, m2/ln/exp on ACT,
    g/beta folded into K<=1/2 broadcast matmuls, 2-op DVE tail per tile.
  - expS/Vh/G/W2 in bf16 (same PE rate, half the SBUF) which buys full
    cross-batch double buffering (bufs=2) to keep the PE fed.
"""

import functools

import numpy as np
import ml_dtypes

import concourse.bass as bass
import concourse.mybir as mybir
import concourse.tile as tile
from concourse import bacc
from concourse import hw_specs as _hw_specs
from concourse.bass_utils import run_bass_kernel_spmd

# The act-table chooser greedily picks the first table containing the needed
# function, so an Exp..Ln..Exp sequence ping-pongs between `exp_and_others`
# and `natural_log` (9 table loads per batch, ~1.5us each). Empty every table
# except the two we want so exp/ln/square/copy all resolve to
# `natural_log_exp_and_others` (ids keep their canonical positions).
_KEEP_TABLES = ("natural_log_exp_and_others", "gelu_and_others")
_orig_get_tables = _hw_specs.get_activation_tables


@functools.cache
def _patched_get_tables(arch):
    tabs = _orig_get_tables(arch)
    return {k: (v if k in _KEEP_TABLES else set()) for k, v in tabs.items()}


_hw_specs.get_activation_tables = _patched_get_tables
bacc.get_activation_tables = _patched_get_tables

B, S, D, H, DH, DFF = 32, 512, 256, 8, 32, 2048
NCORES = 8
BL = B // NCORES
P = 128
DT = D // P     # 2 feature tiles
FT = DFF // P   # 16 ffn tiles
ST = S // P     # 4 seq tiles
f32 = mybir.dt.float32
f32r = mybir.dt.float32r
bf16 = mybir.dt.bfloat16
AF = mybir.ActivationFunctionType
ALU = mybir.AluOpType
EPS = 1e-5


def build_nc():
    nc = bacc.Bacc("TRN2", target_bir_lowering=False, debug=False,
                   num_devices=NCORES)

    QT = nc.dram_tensor("QT", (BL, P, DT, S), bf16, kind="ExternalInput")
    KT = nc.dram_tensor("KT", (BL, P, DT, S), bf16, kind="ExternalInput")
    pT = nc.dram_tensor("pT", (BL, 4, S), bf16, kind="ExternalInput")
    Wq = nc.dram_tensor("Wq", (P, DT, D), bf16, kind="ExternalInput")
    Wv = nc.dram_tensor("Wv", (P, DT, D), bf16, kind="ExternalInput")
    WqA = nc.dram_tensor("WqA", (P, DT, 4, P), bf16, kind="ExternalInput")
    WkA = nc.dram_tensor("WkA", (P, DT, 4, P), bf16, kind="ExternalInput")
    WpAq = nc.dram_tensor("WpAq", (4, 4, P), bf16, kind="ExternalInput")
    WpAk = nc.dram_tensor("WpAk", (4, 4, P), bf16, kind="ExternalInput")
    W1 = nc.dram_tensor("W1", (P, DT, DFF), bf16, kind="ExternalInput")
    W2b = nc.dram_tensor("W2b", (P, FT, D), bf16, kind="ExternalInput")
    bq = nc.dram_tensor("bq", (P, DT), f32, kind="ExternalInput")
    bvb = nc.dram_tensor("bvb", (P, D), f32, kind="ExternalInput")
    b1 = nc.dram_tensor("b1", (P, FT), f32, kind="ExternalInput")
    b2 = nc.dram_tensor("b2", (P, DT), f32, kind="ExternalInput")
    g0r = nc.dram_tensor("g0r", (1, D), f32r, kind="ExternalInput")
    nb0 = nc.dram_tensor("nb0", (1, D), f32r, kind="ExternalInput")
    g1r = nc.dram_tensor("g1r", (1, D), f32r, kind="ExternalInput")
    nb1 = nc.dram_tensor("nb1", (1, D), f32r, kind="ExternalInput")
    one33 = nc.dram_tensor("one33", (P, 2, 33), f32r, kind="ExternalInput")
    Ed4 = nc.dram_tensor("Ed4", (P, 4, 4), bf16, kind="ExternalInput")
    EB4 = nc.dram_tensor("EB4", (4, P), f32r, kind="ExternalInput")
    onesS = nc.dram_tensor("onesS", (1, S), f32r, kind="ExternalInput")
    outT = nc.dram_tensor("outT", (BL, P, DT, S), f32, kind="ExternalOutput")

    with tile.TileContext(nc) as tc:
        with (
            tc.tile_pool(name="singles", bufs=1) as singles,
            tc.tile_pool(name="dbl", bufs=2) as dbl,
            tc.tile_pool(name="ps_mm", bufs=3, space="PSUM") as ps_mm,
            tc.tile_pool(name="ps_acc", bufs=1, space="PSUM") as ps_acc,
            tc.tile_pool(name="ps_av", bufs=2, space="PSUM") as ps_av,
            tc.tile_pool(name="ps_bc", bufs=2, space="PSUM") as ps_bc,
        ):
            def load(dram, shape):
                t = singles.tile(list(shape), dram.dtype, name="w_" + dram.name)
                nc.sync.dma_start(t, dram[tuple(slice(None) for _ in shape)])
                return t

            # order matters: proj weights first so batch 0 starts early
            Wq_sb = load(Wq, (P, DT, D))
            Wv_sb = load(Wv, (P, DT, D))
            WqA_sb = load(WqA, (P, DT, 4, P))
            WkA_sb = load(WkA, (P, DT, 4, P))
            WpAq_sb = load(WpAq, (4, 4, P))
            WpAk_sb = load(WpAk, (4, 4, P))
            Ed4_sb = load(Ed4, (P, 4, 4))
            EB4_sb = load(EB4, (4, P))
            one33_sb = load(one33, (P, 2, 33))
            onesS_sb = load(onesS, (1, S))
            g0_sb = load(g0r, (1, D))
            nb0_sb = load(nb0, (1, D))
            g1_sb = load(g1r, (1, D))
            nb1_sb = load(nb1, (1, D))

            def loadj(dram, shape):
                # stage through DVE so TensorScalar-ish consumers get a
                # same-engine dep (few sync-wait slots on those structs)
                st = load(dram, shape)
                t = singles.tile(list(shape), f32, name="j_" + dram.name)
                nc.vector.tensor_copy(t, st)
                return t

            bq_sb = loadj(bq, (P, DT))
            bvb_sb = loadj(bvb, (P, D))
            b1_sb = loadj(b1, (P, FT))
            b2_sb = loadj(b2, (P, DT))

            W1_sb = load(W1, (P, DT, DFF))
            W2_sb = load(W2b, (P, FT, D))

            eps1 = singles.tile([1, 1], f32)
            nc.vector.memset(eps1, EPS)
            neghalf = singles.tile([1, 1], f32)
            nc.vector.memset(neghalf, -0.5)
            dummy = singles.tile([1, 1], f32)
            nc.vector.memset(dummy, 1.0)

            def layer_norm(x_sb, grow, nbrow, out_sb):
                """out = LN(x) * g + beta.  x_sb [P,DT,S] f32r."""
                x2 = dbl.tile([P, DT, S], f32r, tag="x2", bufs=1, name="x2")
                for t in range(DT):
                    nc.vector.tensor_mul(x2[:, t, :], x_sb[:, t, :],
                                         x_sb[:, t, :])
                # partition 0 <- mean, partition 32 <- E[x^2]
                acc = ps_acc.tile([33, S], f32, tag="acc", name="acc")
                for t in range(DT):
                    nc.tensor.matmul(acc, one33_sb[:, 0, :], x_sb[:, t, :],
                                     start=(t == 0), stop=False)
                for t in range(DT):
                    nc.tensor.matmul(acc, one33_sb[:, 1, :], x2[:, t, :],
                                     start=False, stop=(t == DT - 1))
                rstd = dbl.tile([1, S], f32r, tag="rstd", name="rstd")
                m2v = dbl.tile([1, S], f32r, tag="m2v", name="m2v")
                cst = dbl.tile([1, S], f32r, tag="cst", name="cst")
                nc.scalar.activation(m2v, acc[0:1, :], AF.Square)
                nc.vector.tensor_sub(m2v, acc[32:33, :], m2v)
                nc.scalar.activation(acc[32:33, :], m2v, AF.Ln, bias=eps1)
                # rstd = exp(-0.5*ln(var+eps))
                nc.scalar.activation(rstd, acc[32:33, :], AF.Exp,
                                     scale=neghalf)
                # C = mean * rstd
                nc.vector.tensor_mul(cst, acc[0:1, :], rstd)
                layer_norm.rstd = rstd
                for t in range(DT):
                    bcA = ps_bc.tile([P, S], f32, tag="bc", name="bcA")
                    nc.tensor.matmul(bcA, grow[0:1, t * P:(t + 1) * P],
                                     rstd, start=True, stop=True)
                    bcC = ps_bc.tile([P, S], f32, tag="bc", name="bcC")
                    nc.tensor.matmul(bcC, grow[0:1, t * P:(t + 1) * P],
                                     cst, start=True, stop=False)
                    nc.tensor.matmul(bcC, nbrow[0:1, t * P:(t + 1) * P],
                                     onesS_sb, start=False, stop=True)
                    # out = x*(g*rstd) - (g*mean*rstd - beta)
                    nc.vector.tensor_mul(out_sb[:, t, :], x_sb[:, t, :], bcA)
                    nc.vector.tensor_sub(out_sb[:, t, :], out_sb[:, t, :], bcC)

            def stage_load_proj(b, stt):
                # ---- input loads + projections ----
                QT_sb = dbl.tile([P, DT, S], bf16, tag="qt", name="QT_sb")
                nc.sync.dma_start(QT_sb, QT[b])
                KT_sb = dbl.tile([P, DT, S], bf16, tag="kt", name="KT_sb")
                nc.sync.dma_start(KT_sb, KT[b])
                pT_sb = dbl.tile([4, S], bf16, tag="pt", name="pT_sb")
                nc.sync.dma_start(pT_sb, pT[b])

                # natural Qh (for the attention residual)
                Qh = dbl.tile([P, DT, S], bf16, tag="qh", name="Qh")
                for t in range(DT):
                    ps = ps_mm.tile([P, S], f32, tag="mm", name="psq")
                    for kt in range(DT):
                        nc.tensor.matmul(
                            ps, Wq_sb[:, kt, t * P:(t + 1) * P],
                            QT_sb[:, kt, :],
                            start=(kt == 0), stop=(kt == DT - 1))
                    nc.vector.tensor_tensor(
                        Qh[:, t, :], ps,
                        bq_sb[:, t:t + 1].to_broadcast((P, S)), ALU.add)
                # aug tiles for scores: tile j partitions =
                # [Qh(2j) | Ph(2j) | Qh(2j+1) | Ph(2j+1)], biases folded via
                # the pT ones-row, so one K=64 matmul per (head, kt) yields
                # QK^T + PP^T in a single accumulation
                QA = dbl.tile([P, 4, S], bf16, tag="qa", name="QA")
                KA = dbl.tile([P, 4, S], bf16, tag="ka", name="KA")
                for j in range(4):
                    ps = ps_mm.tile([P, S], f32, tag="mm", name="psqa")
                    for kt in range(DT):
                        nc.tensor.matmul(ps, WqA_sb[:, kt, j, :],
                                         QT_sb[:, kt, :],
                                         start=(kt == 0), stop=False)
                    nc.tensor.matmul(ps, WpAq_sb[:, j, :], pT_sb,
                                     start=False, stop=True)
                    nc.vector.tensor_copy(QA[:, j, :], ps)
                    ps = ps_mm.tile([P, S], f32, tag="mm", name="pska")
                    for kt in range(DT):
                        nc.tensor.matmul(ps, WkA_sb[:, kt, j, :],
                                         KT_sb[:, kt, :],
                                         start=(kt == 0), stop=False)
                    nc.tensor.matmul(ps, WpAk_sb[:, j, :], pT_sb,
                                     start=False, stop=True)
                    nc.vector.tensor_copy(KA[:, j, :], ps)

                # V in natural layout [keys, feat], bf16, bias fused in move
                Vh = dbl.tile([P, ST, D], bf16, tag="vh", name="Vh")
                for st in range(ST):
                    ps = ps_mm.tile([P, S], f32, tag="mm", name="psv")
                    for kt in range(DT):
                        nc.tensor.matmul(
                            ps[:, :D], KT_sb[:, kt, st * P:(st + 1) * P],
                            Wv_sb[:, kt, :],
                            start=(kt == 0), stop=(kt == DT - 1))
                    nc.vector.tensor_add(Vh[:, st, :], ps[:, :D], bvb_sb)
                stt.update(Qh=Qh, QA=QA, KA=KA, Vh=Vh)

            def stage_attn(b, stt):
                Qh, QA, KA, Vh = stt["Qh"], stt["QA"], stt["KA"], stt["Vh"]
                OT = dbl.tile([P, DT, S], f32r, tag="ot", name="OT")
                for quad in range(2):
                    expS = [dbl.tile([P, ST, S], bf16, tag=f"e{i}",
                                     name=f"expS{i}") for i in range(4)]
                    den = ps_acc.tile([4, S], f32, tag="acc", name="den")
                    av = ps_av.tile([P, S], f32, tag="av", name="av")
                    sc_ps = {}
                    for kt in range(ST):
                        for h4 in range(4):
                            base = 64 * (h4 % 2)
                            j = 2 * quad + h4 // 2
                            ps = ps_mm.tile([P, S], f32, tag="mm", name="pssc")
                            sc_ps[h4] = ps
                            nc.tensor.matmul(
                                ps,
                                KA[base:base + 64, j, kt * P:(kt + 1) * P],
                                QA[base:base + 64, j, :],
                                start=True, stop=True,
                                tile_position=(base, 0))
                        for h4 in range(4):
                            nc.scalar.activation(expS[h4][:, kt, :],
                                                 sc_ps[h4], AF.Exp)
                        for h4 in range(4):
                            h = 4 * quad + h4
                            nc.tensor.matmul(
                                den, Ed4_sb[:, h4, :], expS[h4][:, kt, :],
                                start=(kt == 0 and h4 == 0),
                                stop=(kt == ST - 1 and h4 == 3),
                                skip_group_check=True)
                            nc.tensor.matmul(
                                av[32 * h4:32 * h4 + 32, :],
                                Vh[:, kt, 32 * h:32 * h + 32],
                                expS[h4][:, kt, :],
                                start=(kt == 0), stop=(kt == ST - 1),
                                tile_position=(0, 32 * h4),
                                skip_group_check=True)

                    # bc = broadcast(1/den): recip fused into the move
                    r4f = dbl.tile([4, S], f32, tag="r4f", name="r4f")
                    nc.vector.reciprocal_approx_fast(r4f, den[0:4, :])
                    r4 = dbl.tile([4, S], f32r, tag="r4", name="r4")
                    nc.vector.tensor_copy(r4, r4f)
                    bc = ps_bc.tile([P, S], f32, tag="bc", name="bc")
                    nc.tensor.matmul(bc, EB4_sb, r4, start=True, stop=True)
                    bcS = dbl.tile([P, S], f32, tag="bcs", name="bcS")
                    nc.vector.tensor_copy(bcS, bc)
                    nc.vector.tensor_mul(OT[:, quad, :], av, bcS)
                    nc.vector.tensor_add(OT[:, quad, :], OT[:, quad, :],
                                         Qh[:, quad, :])
                stt["OT"] = OT

            def stage_ffn(b, stt):
                OT = stt["OT"]
                LN1 = dbl.tile([P, DT, S], bf16, tag="ln1", name="LN1")
                layer_norm(OT, g0_sb, nb0_sb, LN1)
                # prefetch the gelu table; input dep on LN1's rstd pins this
                # after LN1's Exp in the ACT queue (scheduler can't hoist it)
                nc.scalar.activation(dummy, layer_norm.rstd[0:1, 0:1],
                                     AF.Gelu)

                G = dbl.tile([P, FT, S], bf16, tag="g", bufs=1, name="G")
                for ft in range(FT):
                    ps = ps_mm.tile([P, S], f32, tag="mm", name="psf")
                    for t in range(DT):
                        nc.tensor.matmul(
                            ps, W1_sb[:, t, ft * P:(ft + 1) * P],
                            LN1[:, t, :],
                            start=(t == 0), stop=(t == DT - 1))
                    nc.scalar.activation(G[:, ft, :], ps, AF.Gelu,
                                         bias=b1_sb[:, ft:ft + 1])
                # prefetch the ln/exp table; dep on the last gelu's output
                # pins it after the gelu loop in the ACT queue
                nc.scalar.activation(dummy, G[0:1, FT - 1, 0:1], AF.Ln)
                Z = dbl.tile([P, DT, S], f32r, tag="z", bufs=1, name="Z")
                for t in range(DT):
                    ps = ps_mm.tile([P, S], f32, tag="mm", name="psf2")
                    for ft in range(FT):
                        nc.tensor.matmul(
                            ps, W2_sb[:, ft, t * P:(t + 1) * P],
                            G[:, ft, :],
                            start=(ft == 0), stop=(ft == FT - 1))
                    nc.vector.tensor_add(Z[:, t, :], ps, LN1[:, t, :])
                    nc.vector.tensor_tensor(
                        Z[:, t, :], Z[:, t, :],
                        b2_sb[:, t:t + 1].to_broadcast((P, S)), ALU.add)
                stt["Z"] = Z

            def stage_out(b, stt):
                OUT = dbl.tile([P, DT, S], f32, tag="out", name="OUT")
                layer_norm(stt["Z"], g1_sb, nb1_sb, OUT)
                for t in range(DT):
                    nc.sync.dma_start(outT[b][:, t, :], OUT[:, t, :])

            # software pipeline: emit batch b+1's projections before
            # batch b's LN1 (fills the LN stats stall on the PE queue) and
            # batch b+1's attention before batch b's LN2
            sts = [dict() for _ in range(BL)]
            stage_load_proj(0, sts[0])
            stage_attn(0, sts[0])
            for b in range(BL):
                if b + 1 < BL:
                    stage_load_proj(b + 1, sts[b + 1])
                stage_ffn(b, sts[b])
                if b + 1 < BL:
                    stage_attn(b + 1, sts[b + 1])
                stage_out(b, sts[b])

    nc.finalize()
    return nc


_NC = None


def kernel(Q, K, p, Wq, bq, Wk, bk, Wv, bv, Wp, bp, g0, beta0, W1, b1, W2, b2,
           g1, beta1):
    global _NC
    if _NC is None:
        _NC = build_nc()

    f = np.float32
    bf = ml_dtypes.bfloat16

    def feat_tiles(x):  # [B, S, D] -> [B, P, DT, S]
        x = np.asarray(x, f).transpose(0, 2, 1).reshape(-1, DT, P, S)
        return np.ascontiguousarray(x.transpose(0, 2, 1, 3))

    def pp(vec, n):  # [n*P] -> [P, n]
        return np.ascontiguousarray(np.asarray(vec, f).reshape(n, P).T)

    def wmat(w, n, m):  # [n*P, m] -> [P, n, m]
        w = np.asarray(w, f).reshape(n, P, m)
        return np.ascontiguousarray(w.transpose(1, 0, 2))

    QTf = feat_tiles(Q)
    KTf = feat_tiles(K)
    # p padded to 4 channels; row 3 = ones (carries the PE-proj bias).
    # PE projection pre-scaled by 1/4 so PhPh^T carries the 1/sqrt(DV)=1/16.
    pTf = np.zeros((B, 4, S), f)
    pTf[:, :3, :] = np.transpose(np.asarray(p, f), (0, 2, 1))
    pTf[:, 3, :] = 1.0
    # aug score weights: out tile j partitions =
    # [Qh(2j) | Ph(2j) | Qh(2j+1) | Ph(2j+1)]; pT row3==1 carries biases;
    # PE term pre-scaled by 1/4 each side so PhPh^T carries 1/sqrt(DV)=1/16
    Wq_f = np.asarray(Wq, f)
    Wk_f = np.asarray(Wk, f)
    Wp_f = np.asarray(Wp, f) * 0.25
    bq_f = np.asarray(bq, f)
    bk_f = np.asarray(bk, f)
    bp_f = np.asarray(bp, f) * 0.25

    def aug_w(W):  # [D, D] -> [P, DT, 4, P] lhsT tiles
        out = np.zeros((P, DT, 4, P), f)
        Wt = W.reshape(DT, P, D)  # [kt, row, out_feature]
        for j in range(4):
            for hh in range(2):
                h = 2 * j + hh
                out[:, :, j, 64 * hh:64 * hh + 32] = \
                    Wt[:, :, 32 * h:32 * h + 32].transpose(1, 0, 2)
        return out

    def aug_p(bias):  # [4, 4, P]: rows 0-2 Wp at P slots, row 3 biases
        out = np.zeros((4, 4, P), f)
        for j in range(4):
            for hh in range(2):
                h = 2 * j + hh
                out[:3, j, 64 * hh + 32:64 * hh + 64] = \
                    Wp_f[:, 32 * h:32 * h + 32]
                out[3, j, 64 * hh:64 * hh + 32] = bias[32 * h:32 * h + 32]
                out[3, j, 64 * hh + 32:64 * hh + 64] = \
                    bp_f[32 * h:32 * h + 32]
        return out

    # EB4: r4 row h4 -> out partitions 32*h4..32*h4+31
    EB4m = np.zeros((4, P), f)
    for h4 in range(4):
        EB4m[h4, 32 * h4:32 * h4 + 32] = 1.0
    # Ed4[:, h4, :]: all-ones col h4 (masked partition-sum lhsT)
    Ed4m = np.zeros((P, 4, 4), f)
    for h4 in range(4):
        Ed4m[:, h4, h4] = 1.0
    # LN partition-sum weights (1/D folded in): [:,0,:] puts sum(x)/D at
    # out partition 0, [:,1,:] puts sum(x^2)/D at out partition 32
    one33m = np.zeros((P, 2, 33), f)
    one33m[:, 0, 0] = 1.0 / D
    one33m[:, 1, 32] = 1.0 / D

    shared = {
        "Wq": wmat(Wq, DT, D).astype(bf),
        "Wv": wmat(Wv, DT, D).astype(bf),
        "WqA": aug_w(Wq_f).astype(bf), "WkA": aug_w(Wk_f).astype(bf),
        "WpAq": aug_p(bq_f).astype(bf), "WpAk": aug_p(bk_f).astype(bf),
        "W1": wmat(W1, DT, DFF).astype(bf),
        "W2b": wmat(W2, FT, D).astype(bf),
        "bq": pp(bq, DT),
        "bvb": np.ascontiguousarray(np.broadcast_to(np.asarray(bv, f), (P, D))),
        "b1": pp(b1, FT), "b2": pp(b2, DT),
        "g0r": np.asarray(g0, f).reshape(1, D),
        "nb0": -np.asarray(beta0, f).reshape(1, D),
        "g1r": np.asarray(g1, f).reshape(1, D),
        "nb1": -np.asarray(beta1, f).reshape(1, D),
        "one33": one33m,
        "Ed4": Ed4m.astype(bf), "EB4": EB4m,
        "onesS": np.ones((1, S), f),
    }
    in_maps = []
    for c in range(NCORES):
        m = dict(shared)
        m["QT"] = np.ascontiguousarray(QTf[c * BL:(c + 1) * BL]).astype(bf)
        m["KT"] = np.ascontiguousarray(KTf[c * BL:(c + 1) * BL]).astype(bf)
        m["pT"] = np.ascontiguousarray(pTf[c * BL:(c + 1) * BL]).astype(bf)
        in_maps.append(m)

    import os
    trace = bool(os.environ.get("BASS_TRACE"))
    res = run_bass_kernel_spmd(_NC, in_maps, core_ids=list(range(NCORES)),
                               trace=trace)
    kernel._LAST = res
    outs = [res.results[c]["outT"] for c in range(NCORES)]
    full = np.concatenate(outs, axis=0)  # [B, P, DT, S]
    full = full.transpose(0, 2, 1, 3).reshape(B, D, S)  # [B, D, S]
    return np.ascontiguousarray(full.transpose(0, 2, 1))


# revision 27
# speedup vs baseline: 1.4019x; 1.0035x over previous
"""Trainium2 Bass kernel for nn_MAB_2121713844542 (dense transformer block).

Data-parallel over batch B=32 across 8 cores (4 batches/core), activations
transposed [feature, seq] so every matmul contracts on partitions.

v2 layout/engine plan (vs baseline):
  - softmax denominators accumulate into one [4,S] PSUM tile per quad via
    masked-ones matmuls; 1/d = exp(-ln(d)) on the scalar engine (exp/ln
    tables stay resident) -- removes the 107us of DVE RECIPROCAL.
  - AV matmuls write natural head positions (tile_position col=32*h4) so
    the softmax divide + Qh residual are two full-width DVE ops per quad.
  - LayerNorm: 1/D folded into the ones-matmul weights, m2/ln/exp on ACT,
    g/beta folded into K<=1/2 broadcast matmuls, 2-op DVE tail per tile.
  - expS/Vh/G/W2 in bf16 (same PE rate, half the SBUF) which buys full
    cross-batch double buffering (bufs=2) to keep the PE fed.
"""

import functools

import numpy as np
import ml_dtypes

import concourse.bass as bass
import concourse.mybir as mybir
import concourse.tile as tile
from concourse import bacc
from concourse import hw_specs as _hw_specs
from concourse.bass_utils import run_bass_kernel_spmd

# The act-table chooser greedily picks the first table containing the needed
# function, so an Exp..Ln..Exp sequence ping-pongs between `exp_and_others`
# and `natural_log` (9 table loads per batch, ~1.5us each). Empty every table
# except the two we want so exp/ln/square/copy all resolve to
# `natural_log_exp_and_others` (ids keep their canonical positions).
_KEEP_TABLES = ("natural_log_exp_and_others", "gelu_and_others")
_orig_get_tables = _hw_specs.get_activation_tables


@functools.cache
def _patched_get_tables(arch):
    tabs = _orig_get_tables(arch)
    return {k: (v if k in _KEEP_TABLES else set()) for k, v in tabs.items()}


_hw_specs.get_activation_tables = _patched_get_tables
bacc.get_activation_tables = _patched_get_tables

B, S, D, H, DH, DFF = 32, 512, 256, 8, 32, 2048
NCORES = 8
BL = B // NCORES
P = 128
DT = D // P     # 2 feature tiles
FT = DFF // P   # 16 ffn tiles
ST = S // P     # 4 seq tiles
f32 = mybir.dt.float32
f32r = mybir.dt.float32r
bf16 = mybir.dt.bfloat16
AF = mybir.ActivationFunctionType
ALU = mybir.AluOpType
EPS = 1e-5


def build_nc():
    nc = bacc.Bacc("TRN2", target_bir_lowering=False, debug=False,
                   num_devices=NCORES)

    QT = nc.dram_tensor("QT", (BL, P, DT, S), bf16, kind="ExternalInput")
    KT = nc.dram_tensor("KT", (BL, P, DT, S), bf16, kind="ExternalInput")
    pT = nc.dram_tensor("pT", (BL, 4, S), bf16, kind="ExternalInput")
    Wq = nc.dram_tensor("Wq", (P, DT, D), bf16, kind="ExternalInput")
    Wv = nc.dram_tensor("Wv", (P, DT, D), bf16, kind="ExternalInput")
    WqA = nc.dram_tensor("WqA", (P, DT, 4, P), bf16, kind="ExternalInput")
    WkA = nc.dram_tensor("WkA", (P, DT, 4, P), bf16, kind="ExternalInput")
    WpAq = nc.dram_tensor("WpAq", (4, 4, P), bf16, kind="ExternalInput")
    WpAk = nc.dram_tensor("WpAk", (4, 4, P), bf16, kind="ExternalInput")
    W1 = nc.dram_tensor("W1", (P, DT, DFF), bf16, kind="ExternalInput")
    W2b = nc.dram_tensor("W2b", (P, FT, D), bf16, kind="ExternalInput")
    bq = nc.dram_tensor("bq", (P, DT), f32, kind="ExternalInput")
    bvb = nc.dram_tensor("bvb", (P, D), f32, kind="ExternalInput")
    b1 = nc.dram_tensor("b1", (P, FT), f32, kind="ExternalInput")
    b2 = nc.dram_tensor("b2", (P, DT), f32, kind="ExternalInput")
    g0r = nc.dram_tensor("g0r", (1, D), f32r, kind="ExternalInput")
    nb0 = nc.dram_tensor("nb0", (1, D), f32r, kind="ExternalInput")
    g1r = nc.dram_tensor("g1r", (1, D), f32r, kind="ExternalInput")
    nb1 = nc.dram_tensor("nb1", (1, D), f32r, kind="ExternalInput")
    one33 = nc.dram_tensor("one33", (P, 2, 33), f32r, kind="ExternalInput")
    Ed4 = nc.dram_tensor("Ed4", (P, 4, 4), bf16, kind="ExternalInput")
    EB4 = nc.dram_tensor("EB4", (4, P), f32r, kind="ExternalInput")
    onesS = nc.dram_tensor("onesS", (1, S), f32r, kind="ExternalInput")
    outT = nc.dram_tensor("outT", (BL, P, DT, S), f32, kind="ExternalOutput")

    with tile.TileContext(nc) as tc:
        with (
            tc.tile_pool(name="singles", bufs=1) as singles,
            tc.tile_pool(name="dbl", bufs=2) as dbl,
            tc.tile_pool(name="ps_mm", bufs=3, space="PSUM") as ps_mm,
            tc.tile_pool(name="ps_acc", bufs=1, space="PSUM") as ps_acc,
            tc.tile_pool(name="ps_av", bufs=2, space="PSUM") as ps_av,
            tc.tile_pool(name="ps_bc", bufs=2, space="PSUM") as ps_bc,
        ):
            def load(dram, shape):
                t = singles.tile(list(shape), dram.dtype, name="w_" + dram.name)
                nc.sync.dma_start(t, dram[tuple(slice(None) for _ in shape)])
                return t

            # order matters: proj weights first so batch 0 starts early
            Wq_sb = load(Wq, (P, DT, D))
            Wv_sb = load(Wv, (P, DT, D))
            WqA_sb = load(WqA, (P, DT, 4, P))
            WkA_sb = load(WkA, (P, DT, 4, P))
            WpAq_sb = load(WpAq, (4, 4, P))
            WpAk_sb = load(WpAk, (4, 4, P))
            Ed4_sb = load(Ed4, (P, 4, 4))
            EB4_sb = load(EB4, (4, P))
            one33_sb = load(one33, (P, 2, 33))
            onesS_sb = load(onesS, (1, S))
            g0_sb = load(g0r, (1, D))
            nb0_sb = load(nb0, (1, D))
            g1_sb = load(g1r, (1, D))
            nb1_sb = load(nb1, (1, D))

            def loadj(dram, shape):
                # stage through DVE so TensorScalar-ish consumers get a
                # same-engine dep (few sync-wait slots on those structs)
                st = load(dram, shape)
                t = singles.tile(list(shape), f32, name="j_" + dram.name)
                nc.vector.tensor_copy(t, st)
                return t

            bq_sb = loadj(bq, (P, DT))
            bvb_sb = loadj(bvb, (P, D))
            b1_sb = loadj(b1, (P, FT))
            b2_sb = loadj(b2, (P, DT))

            W1_sb = load(W1, (P, DT, DFF))
            W2_sb = load(W2b, (P, FT, D))

            eps1 = singles.tile([1, 1], f32)
            nc.vector.memset(eps1, EPS)
            neghalf = singles.tile([1, 1], f32)
            nc.vector.memset(neghalf, -0.5)
            dummy = singles.tile([1, 1], f32)
            nc.vector.memset(dummy, 1.0)

            def layer_norm(x_sb, grow, nbrow, out_sb):
                """out = LN(x) * g + beta.  x_sb [P,DT,S] f32r."""
                x2 = dbl.tile([P, DT, S], f32r, tag="x2", bufs=1, name="x2")
                for t in range(DT):
                    nc.vector.tensor_mul(x2[:, t, :], x_sb[:, t, :],
                                         x_sb[:, t, :])
                # partition 0 <- mean, partition 32 <- E[x^2]
                acc = ps_acc.tile([33, S], f32, tag="acc", name="acc")
                for t in range(DT):
                    nc.tensor.matmul(acc, one33_sb[:, 0, :], x_sb[:, t, :],
                                     start=(t == 0), stop=False)
                for t in range(DT):
                    nc.tensor.matmul(acc, one33_sb[:, 1, :], x2[:, t, :],
                                     start=False, stop=(t == DT - 1))
                rstd = dbl.tile([1, S], f32r, tag="rstd", name="rstd")
                m2v = dbl.tile([1, S], f32r, tag="m2v", name="m2v")
                cst = dbl.tile([1, S], f32r, tag="cst", name="cst")
                nc.scalar.activation(m2v, acc[0:1, :], AF.Square)
                nc.vector.tensor_sub(m2v, acc[32:33, :], m2v)
                nc.scalar.activation(acc[32:33, :], m2v, AF.Ln, bias=eps1)
                # rstd = exp(-0.5*ln(var+eps))
                nc.scalar.activation(rstd, acc[32:33, :], AF.Exp,
                                     scale=neghalf)
                # C = mean * rstd
                nc.vector.tensor_mul(cst, acc[0:1, :], rstd)
                layer_norm.rstd = rstd
                for t in range(DT):
                    bcA = ps_bc.tile([P, S], f32, tag="bc", name="bcA")
                    nc.tensor.matmul(bcA, grow[0:1, t * P:(t + 1) * P],
                                     rstd, start=True, stop=True)
                    bcC = ps_bc.tile([P, S], f32, tag="bc", name="bcC")
                    nc.tensor.matmul(bcC, grow[0:1, t * P:(t + 1) * P],
                                     cst, start=True, stop=False)
                    nc.tensor.matmul(bcC, nbrow[0:1, t * P:(t + 1) * P],
                                     onesS_sb, start=False, stop=True)
                    # out = x*(g*rstd) - (g*mean*rstd - beta)
                    nc.vector.tensor_mul(out_sb[:, t, :], x_sb[:, t, :], bcA)
                    nc.vector.tensor_sub(out_sb[:, t, :], out_sb[:, t, :], bcC)

            def stage_load_proj(b, stt):
                # ---- input loads + projections ----
                QT_sb = dbl.tile([P, DT, S], bf16, tag="qt", name="QT_sb")
                nc.sync.dma_start(QT_sb, QT[b])
                KT_sb = dbl.tile([P, DT, S], bf16, tag="kt", name="KT_sb")
                nc.sync.dma_start(KT_sb, KT[b])
                pT_sb = dbl.tile([4, S], bf16, tag="pt", name="pT_sb")
                nc.sync.dma_start(pT_sb, pT[b])

                # natural Qh (for the attention residual)
                Qh = dbl.tile([P, DT, S], bf16, tag="qh", name="Qh")
                for t in range(DT):
                    ps = ps_mm.tile([P, S], f32, tag="mm", name="psq")
                    for kt in range(DT):
                        nc.tensor.matmul(
                            ps, Wq_sb[:, kt, t * P:(t + 1) * P],
                            QT_sb[:, kt, :],
                            start=(kt == 0), stop=(kt == DT - 1))
                    nc.vector.tensor_tensor(
                        Qh[:, t, :], ps,
                        bq_sb[:, t:t + 1].to_broadcast((P, S)), ALU.add)
                # aug tiles for scores: tile j partitions =
                # [Qh(2j) | Ph(2j) | Qh(2j+1) | Ph(2j+1)], biases folded via
                # the pT ones-row, so one K=64 matmul per (head, kt) yields
                # QK^T + PP^T in a single accumulation
                QA = dbl.tile([P, 4, S], bf16, tag="qa", name="QA")
                KA = dbl.tile([P, 4, S], bf16, tag="ka", name="KA")
                for j in range(4):
                    ps = ps_mm.tile([P, S], f32, tag="mm", name="psqa")
                    for kt in range(DT):
                        nc.tensor.matmul(ps, WqA_sb[:, kt, j, :],
                                         QT_sb[:, kt, :],
                                         start=(kt == 0), stop=False)
                    nc.tensor.matmul(ps, WpAq_sb[:, j, :], pT_sb,
                                     start=False, stop=True)
                    nc.vector.tensor_copy(QA[:, j, :], ps)
                    ps = ps_mm.tile([P, S], f32, tag="mm", name="pska")
                    for kt in range(DT):
                        nc.tensor.matmul(ps, WkA_sb[:, kt, j, :],
                                         KT_sb[:, kt, :],
                                         start=(kt == 0), stop=False)
                    nc.tensor.matmul(ps, WpAk_sb[:, j, :], pT_sb,
                                     start=False, stop=True)
                    nc.vector.tensor_copy(KA[:, j, :], ps)

                # V in natural layout [keys, feat], bf16, bias fused in move
                Vh = dbl.tile([P, ST, D], bf16, tag="vh", name="Vh")
                for st in range(ST):
                    ps = ps_mm.tile([P, S], f32, tag="mm", name="psv")
                    for kt in range(DT):
                        nc.tensor.matmul(
                            ps[:, :D], KT_sb[:, kt, st * P:(st + 1) * P],
                            Wv_sb[:, kt, :],
                            start=(kt == 0), stop=(kt == DT - 1))
                    nc.vector.tensor_add(Vh[:, st, :], ps[:, :D], bvb_sb)
                stt.update(Qh=Qh, QA=QA, KA=KA, Vh=Vh)

            def stage_attn(b, stt):
                Qh, QA, KA, Vh = stt["Qh"], stt["QA"], stt["KA"], stt["Vh"]
                OT = dbl.tile([P, DT, S], f32r, tag="ot", name="OT")
                for quad in range(2):
                    expS = [dbl.tile([P, ST, S], bf16, tag=f"e{i}",
                                     name=f"expS{i}") for i in range(4)]
                    den = ps_acc.tile([4, S], f32, tag="acc", name="den")
                    av = ps_av.tile([P, S], f32, tag="av", name="av")
                    sc_ps = {}
                    for kt in range(ST):
                        for h4 in range(4):
                            base = 64 * (h4 % 2)
                            j = 2 * quad + h4 // 2
                            ps = ps_mm.tile([P, S], f32, tag="mm", name="pssc")
                            sc_ps[h4] = ps
                            nc.tensor.matmul(
                                ps,
                                KA[base:base + 64, j, kt * P:(kt + 1) * P],
                                QA[base:base + 64, j, :],
                                start=True, stop=True,
                                tile_position=(base, 0))
                        for h4 in range(4):
                            nc.scalar.activation(expS[h4][:, kt, :],
                                                 sc_ps[h4], AF.Exp)
                        for h4 in range(4):
                            h = 4 * quad + h4
                            nc.tensor.matmul(
                                den, Ed4_sb[:, h4, :], expS[h4][:, kt, :],
                                start=(kt == 0 and h4 == 0),
                                stop=(kt == ST - 1 and h4 == 3),
                                skip_group_check=True)
                            nc.tensor.matmul(
                                av[32 * h4:32 * h4 + 32, :],
                                Vh[:, kt, 32 * h:32 * h + 32],
                                expS[h4][:, kt, :],
                                start=(kt == 0), stop=(kt == ST - 1),
                                tile_position=(0, 32 * h4),
                                skip_group_check=True)

                    # bc = broadcast(1/den): recip fused into the move
                    r4f = dbl.tile([4, S], f32, tag="r4f", name="r4f")
                    nc.vector.reciprocal_approx_fast(r4f, den[0:4, :])
                    r4 = dbl.tile([4, S], f32r, tag="r4", name="r4")
                    nc.vector.tensor_copy(r4, r4f)
                    bc = ps_bc.tile([P, S], f32, tag="bc", name="bc")
                    nc.tensor.matmul(bc, EB4_sb, r4, start=True, stop=True)
                    bcS = dbl.tile([P, S], f32, tag="bcs", name="bcS")
                    nc.vector.tensor_copy(bcS, bc)
                    nc.vector.tensor_mul(OT[:, quad, :], av, bcS)
                    nc.vector.tensor_add(OT[:, quad, :], OT[:, quad, :],
                                         Qh[:, quad, :])
                stt["OT"] = OT

            def stage_ffn(b, stt):
                OT = stt["OT"]
                LN1 = dbl.tile([P, DT, S], bf16, tag="ln1", name="LN1")
                layer_norm(OT, g0_sb, nb0_sb, LN1)
                # prefetch the gelu table; input dep on LN1's rstd pins this
                # after LN1's Exp in the ACT queue (scheduler can't hoist it)
                nc.scalar.activation(dummy, layer_norm.rstd[0:1, 0:1],
                                     AF.Gelu)

                G = dbl.tile([P, FT, S], bf16, tag="g", bufs=1, name="G")
                for ft in range(FT):
                    ps = ps_mm.tile([P, S], f32, tag="mm", name="psf")
                    for t in range(DT):
                        nc.tensor.matmul(
                            ps, W1_sb[:, t, ft * P:(ft + 1) * P],
                            LN1[:, t, :],
                            start=(t == 0), stop=(t == DT - 1))
                    nc.scalar.activation(G[:, ft, :], ps, AF.Gelu,
                                         bias=b1_sb[:, ft:ft + 1])
                # prefetch the ln/exp table; dep on the last gelu's output
                # pins it after the gelu loop in the ACT queue
                nc.scalar.activation(dummy, G[0:1, FT - 1, 0:1], AF.Ln)
                Z = dbl.tile([P, DT, S], f32r, tag="z", bufs=1, name="Z")
                for t in range(DT):
                    ps = ps_mm.tile([P, S], f32, tag="mm", name="psf2")
                    for ft in range(FT):
                        nc.tensor.matmul(
                            ps, W2_sb[:, ft, t * P:(t + 1) * P],
                            G[:, ft, :],
                            start=(ft == 0), stop=(ft == FT - 1))
                    nc.vector.tensor_add(Z[:, t, :], ps, LN1[:, t, :])
                    nc.vector.tensor_tensor(
                        Z[:, t, :], Z[:, t, :],
                        b2_sb[:, t:t + 1].to_broadcast((P, S)), ALU.add)
                stt["Z"] = Z

            def stage_out(b, stt):
                OUT = dbl.tile([P, DT, S], f32, tag="out", name="OUT")
                layer_norm(stt["Z"], g1_sb, nb1_sb, OUT)
                for t in range(DT):
                    nc.sync.dma_start(outT[b][:, t, :], OUT[:, t, :])

            # software pipeline: emit batch b+1's projections before
            # batch b's LN1 (fills the LN stats stall on the PE queue) and
            # batch b+1's attention before batch b's LN2
            sts = [dict() for _ in range(BL)]
            stage_load_proj(0, sts[0])
            stage_attn(0, sts[0])
            for b in range(BL):
                if b + 1 < BL:
                    stage_load_proj(b + 1, sts[b + 1])
                    stage_attn(b + 1, sts[b + 1])
                stage_ffn(b, sts[b])
                stage_out(b, sts[b])

    nc.finalize()
    return nc


_NC = None


def kernel(Q, K, p, Wq, bq, Wk, bk, Wv, bv, Wp, bp, g0, beta0, W1, b1, W2, b2,
           g1, beta1):
    global _NC
    if _NC is None:
        _NC = build_nc()

    f = np.float32
    bf = ml_dtypes.bfloat16

    def feat_tiles(x):  # [B, S, D] -> [B, P, DT, S]
        x = np.asarray(x, f).transpose(0, 2, 1).reshape(-1, DT, P, S)
        return np.ascontiguousarray(x.transpose(0, 2, 1, 3))

    def pp(vec, n):  # [n*P] -> [P, n]
        return np.ascontiguousarray(np.asarray(vec, f).reshape(n, P).T)

    def wmat(w, n, m):  # [n*P, m] -> [P, n, m]
        w = np.asarray(w, f).reshape(n, P, m)
        return np.ascontiguousarray(w.transpose(1, 0, 2))

    QTf = feat_tiles(Q)
    KTf = feat_tiles(K)
    # p padded to 4 channels; row 3 = ones (carries the PE-proj bias).
    # PE projection pre-scaled by 1/4 so PhPh^T carries the 1/sqrt(DV)=1/16.
    pTf = np.zeros((B, 4, S), f)
    pTf[:, :3, :] = np.transpose(np.asarray(p, f), (0, 2, 1))
    pTf[:, 3, :] = 1.0
    # aug score weights: out tile j partitions =
    # [Qh(2j) | Ph(2j) | Qh(2j+1) | Ph(2j+1)]; pT row3==1 carries biases;
    # PE term pre-scaled by 1/4 each side so PhPh^T carries 1/sqrt(DV)=1/16
    Wq_f = np.asarray(Wq, f)
    Wk_f = np.asarray(Wk, f)
    Wp_f = np.asarray(Wp, f) * 0.25
    bq_f = np.asarray(bq, f)
    bk_f = np.asarray(bk, f)
    bp_f = np.asarray(bp, f) * 0.25

    def aug_w(W):  # [D, D] -> [P, DT, 4, P] lhsT tiles
        out = np.zeros((P, DT, 4, P), f)
        Wt = W.reshape(DT, P, D)  # [kt, row, out_feature]
        for j in range(4):
            for hh in range(2):
                h = 2 * j + hh
                out[:, :, j, 64 * hh:64 * hh + 32] = \
                    Wt[:, :, 32 * h:32 * h + 32].transpose(1, 0, 2)
        return out

    def aug_p(bias):  # [4, 4, P]: rows 0-2 Wp at P slots, row 3 biases
        out = np.zeros((4, 4, P), f)
        for j in range(4):
            for hh in range(2):
                h = 2 * j + hh
                out[:3, j, 64 * hh + 32:64 * hh + 64] = \
                    Wp_f[:, 32 * h:32 * h + 32]
                out[3, j, 64 * hh:64 * hh + 32] = bias[32 * h:32 * h + 32]
                out[3, j, 64 * hh + 32:64 * hh + 64] = \
                    bp_f[32 * h:32 * h + 32]
        return out

    # EB4: r4 row h4 -> out partitions 32*h4..32*h4+31
    EB4m = np.zeros((4, P), f)
    for h4 in range(4):
        EB4m[h4, 32 * h4:32 * h4 + 32] = 1.0
    # Ed4[:, h4, :]: all-ones col h4 (masked partition-sum lhsT)
    Ed4m = np.zeros((P, 4, 4), f)
    for h4 in range(4):
        Ed4m[:, h4, h4] = 1.0
    # LN partition-sum weights (1/D folded in): [:,0,:] puts sum(x)/D at
    # out partition 0, [:,1,:] puts sum(x^2)/D at out partition 32
    one33m = np.zeros((P, 2, 33), f)
    one33m[:, 0, 0] = 1.0 / D
    one33m[:, 1, 32] = 1.0 / D

    shared = {
        "Wq": wmat(Wq, DT, D).astype(bf),
        "Wv": wmat(Wv, DT, D).astype(bf),
        "WqA": aug_w(Wq_f).astype(bf), "WkA": aug_w(Wk_f).astype(bf),
        "WpAq": aug_p(bq_f).astype(bf), "WpAk": aug_p(bk_f).astype(bf),
        "W1": wmat(W1, DT, DFF).astype(bf),
        "W2b": wmat(W2, FT, D).astype(bf),
        "bq": pp(bq, DT),
        "bvb": np.ascontiguousarray(np.broadcast_to(np.asarray(bv, f), (P, D))),
        "b1": pp(b1, FT), "b2": pp(b2, DT),
        "g0r": np.asarray(g0, f).reshape(1, D),
        "nb0": -np.asarray(beta0, f).reshape(1, D),
        "g1r": np.asarray(g1, f).reshape(1, D),
        "nb1": -np.asarray(beta1, f).reshape(1, D),
        "one33": one33m,
        "Ed4": Ed4m.astype(bf), "EB4": EB4m,
        "onesS": np.ones((1, S), f),
    }
    in_maps = []
    for c in range(NCORES):
        m = dict(shared)
        m["QT"] = np.ascontiguousarray(QTf[c * BL:(c + 1) * BL]).astype(bf)
        m["KT"] = np.ascontiguousarray(KTf[c * BL:(c + 1) * BL]).astype(bf)
        m["pT"] = np.ascontiguousarray(pTf[c * BL:(c + 1) * BL]).astype(bf)
        in_maps.append(m)

    import os
    trace = bool(os.environ.get("BASS_TRACE"))
    res = run_bass_kernel_spmd(_NC, in_maps, core_ids=list(range(NCORES)),
                               trace=trace)
    kernel._LAST = res
    outs = [res.results[c]["outT"] for c in range(NCORES)]
    full = np.concatenate(outs, axis=0)  # [B, P, DT, S]
    full = full.transpose(0, 2, 1, 3).reshape(B, D, S)  # [B, D, S]
    return np.ascontiguousarray(full.transpose(0, 2, 1))


# revision 28
# speedup vs baseline: 1.4417x; 1.0284x over previous
"""Trainium2 Bass kernel for nn_MAB_2121713844542 (dense transformer block).

Data-parallel over batch B=32 across 8 cores (4 batches/core), activations
transposed [feature, seq] so every matmul contracts on partitions.

v2 layout/engine plan (vs baseline):
  - softmax denominators accumulate into one [4,S] PSUM tile per quad via
    masked-ones matmuls; 1/d = exp(-ln(d)) on the scalar engine (exp/ln
    tables stay resident) -- removes the 107us of DVE RECIPROCAL.
  - AV matmuls write natural head positions (tile_position col=32*h4) so
    the softmax divide + Qh residual are two full-width DVE ops per quad.
  - LayerNorm: 1/D folded into the ones-matmul weights, m2/ln/exp on ACT,
    g/beta folded into K<=1/2 broadcast matmuls, 2-op DVE tail per tile.
  - expS/Vh/G/W2 in bf16 (same PE rate, half the SBUF) which buys full
    cross-batch double buffering (bufs=2) to keep the PE fed.
"""

import functools

import numpy as np
import ml_dtypes

import concourse.bass as bass
import concourse.mybir as mybir
import concourse.tile as tile
from concourse import bacc
from concourse import hw_specs as _hw_specs
from concourse.bass_utils import run_bass_kernel_spmd

# The act-table chooser greedily picks the first table containing the needed
# function, so an Exp..Ln..Exp sequence ping-pongs between `exp_and_others`
# and `natural_log` (9 table loads per batch, ~1.5us each). Empty every table
# except the two we want so exp/ln/square/copy all resolve to
# `natural_log_exp_and_others` (ids keep their canonical positions).
_KEEP_TABLES = ("natural_log_exp_and_others", "gelu_and_others")
_orig_get_tables = _hw_specs.get_activation_tables


@functools.cache
def _patched_get_tables(arch):
    tabs = _orig_get_tables(arch)
    return {k: (v if k in _KEEP_TABLES else set()) for k, v in tabs.items()}


_hw_specs.get_activation_tables = _patched_get_tables
bacc.get_activation_tables = _patched_get_tables

B, S, D, H, DH, DFF = 32, 512, 256, 8, 32, 2048
NCORES = 8
BL = B // NCORES
P = 128
DT = D // P     # 2 feature tiles
FT = DFF // P   # 16 ffn tiles
ST = S // P     # 4 seq tiles
f32 = mybir.dt.float32
f32r = mybir.dt.float32r
bf16 = mybir.dt.bfloat16
AF = mybir.ActivationFunctionType
ALU = mybir.AluOpType
EPS = 1e-5


def build_nc():
    nc = bacc.Bacc("TRN2", target_bir_lowering=False, debug=False,
                   num_devices=NCORES)

    QT = nc.dram_tensor("QT", (BL, P, DT, S), bf16, kind="ExternalInput")
    KT = nc.dram_tensor("KT", (BL, P, DT, S), bf16, kind="ExternalInput")
    pT = nc.dram_tensor("pT", (BL, 4, S), bf16, kind="ExternalInput")
    Wq = nc.dram_tensor("Wq", (P, DT, D), bf16, kind="ExternalInput")
    Wv = nc.dram_tensor("Wv", (P, DT, D), bf16, kind="ExternalInput")
    WqA = nc.dram_tensor("WqA", (P, DT, 4, P), bf16, kind="ExternalInput")
    WkA = nc.dram_tensor("WkA", (P, DT, 4, P), bf16, kind="ExternalInput")
    WpAq = nc.dram_tensor("WpAq", (4, 4, P), bf16, kind="ExternalInput")
    WpAk = nc.dram_tensor("WpAk", (4, 4, P), bf16, kind="ExternalInput")
    W1 = nc.dram_tensor("W1", (P, DT, DFF), bf16, kind="ExternalInput")
    W2b = nc.dram_tensor("W2b", (P, FT, D), bf16, kind="ExternalInput")
    bq = nc.dram_tensor("bq", (P, DT), f32, kind="ExternalInput")
    bvb = nc.dram_tensor("bvb", (P, D), f32, kind="ExternalInput")
    b1 = nc.dram_tensor("b1", (P, FT), f32, kind="ExternalInput")
    b2 = nc.dram_tensor("b2", (P, DT), f32, kind="ExternalInput")
    g0r = nc.dram_tensor("g0r", (1, D), f32r, kind="ExternalInput")
    nb0 = nc.dram_tensor("nb0", (1, D), f32r, kind="ExternalInput")
    g1r = nc.dram_tensor("g1r", (1, D), f32r, kind="ExternalInput")
    nb1 = nc.dram_tensor("nb1", (1, D), f32r, kind="ExternalInput")
    one33 = nc.dram_tensor("one33", (P, 2, 33), f32r, kind="ExternalInput")
    Ed4 = nc.dram_tensor("Ed4", (P, 4, 4), bf16, kind="ExternalInput")
    EB4 = nc.dram_tensor("EB4", (4, P), f32r, kind="ExternalInput")
    onesS = nc.dram_tensor("onesS", (1, S), f32r, kind="ExternalInput")
    outT = nc.dram_tensor("outT", (BL, P, DT, S), f32, kind="ExternalOutput")

    with tile.TileContext(nc) as tc:
        with (
            tc.tile_pool(name="singles", bufs=1) as singles,
            tc.tile_pool(name="dbl", bufs=2) as dbl,
            tc.tile_pool(name="ps_mm", bufs=3, space="PSUM") as ps_mm,
            tc.tile_pool(name="ps_acc", bufs=1, space="PSUM") as ps_acc,
            tc.tile_pool(name="ps_av", bufs=2, space="PSUM") as ps_av,
            tc.tile_pool(name="ps_bc", bufs=2, space="PSUM") as ps_bc,
        ):
            def load(dram, shape):
                t = singles.tile(list(shape), dram.dtype, name="w_" + dram.name)
                nc.sync.dma_start(t, dram[tuple(slice(None) for _ in shape)])
                return t

            # order matters: only what batch 0's proj needs loads first;
            # the 2MB of FFN weights stream in during batch-0 attention
            Wq_sb = load(Wq, (P, DT, D))

            def loadj(dram, shape):
                # stage through DVE so TensorScalar-ish consumers get a
                # same-engine dep (few sync-wait slots on those structs)
                st = load(dram, shape)
                t = singles.tile(list(shape), f32, name="j_" + dram.name)
                nc.vector.tensor_copy(t, st)
                return t

            bq_sb = loadj(bq, (P, DT))

            eps1 = singles.tile([1, 1], f32)
            nc.vector.memset(eps1, EPS)
            neghalf = singles.tile([1, 1], f32)
            nc.vector.memset(neghalf, -0.5)
            dummy = singles.tile([1, 1], f32)
            nc.vector.memset(dummy, 1.0)

            def layer_norm(x_sb, grow, nbrow, out_sb):
                """out = LN(x) * g + beta.  x_sb [P,DT,S] f32r."""
                x2 = dbl.tile([P, DT, S], f32r, tag="x2", bufs=1, name="x2")
                for t in range(DT):
                    nc.vector.tensor_mul(x2[:, t, :], x_sb[:, t, :],
                                         x_sb[:, t, :])
                # partition 0 <- mean, partition 32 <- E[x^2]
                acc = ps_acc.tile([33, S], f32, tag="acc", name="acc")
                for t in range(DT):
                    nc.tensor.matmul(acc, one33_sb[:, 0, :], x_sb[:, t, :],
                                     start=(t == 0), stop=False)
                for t in range(DT):
                    nc.tensor.matmul(acc, one33_sb[:, 1, :], x2[:, t, :],
                                     start=False, stop=(t == DT - 1))
                rstd = dbl.tile([1, S], f32r, tag="rstd", name="rstd")
                m2v = dbl.tile([1, S], f32r, tag="m2v", name="m2v")
                cst = dbl.tile([1, S], f32r, tag="cst", name="cst")
                nc.scalar.activation(m2v, acc[0:1, :], AF.Square)
                nc.vector.tensor_sub(m2v, acc[32:33, :], m2v)
                nc.scalar.activation(acc[32:33, :], m2v, AF.Ln, bias=eps1)
                # rstd = exp(-0.5*ln(var+eps))
                nc.scalar.activation(rstd, acc[32:33, :], AF.Exp,
                                     scale=neghalf)
                # C = mean * rstd
                nc.vector.tensor_mul(cst, acc[0:1, :], rstd)
                layer_norm.rstd = rstd
                for t in range(DT):
                    bcA = ps_bc.tile([P, S], f32, tag="bc", name="bcA")
                    nc.tensor.matmul(bcA, grow[0:1, t * P:(t + 1) * P],
                                     rstd, start=True, stop=True)
                    bcC = ps_bc.tile([P, S], f32, tag="bc", name="bcC")
                    nc.tensor.matmul(bcC, grow[0:1, t * P:(t + 1) * P],
                                     cst, start=True, stop=False)
                    nc.tensor.matmul(bcC, nbrow[0:1, t * P:(t + 1) * P],
                                     onesS_sb, start=False, stop=True)
                    # out = x*(g*rstd) - (g*mean*rstd - beta)
                    nc.vector.tensor_mul(out_sb[:, t, :], x_sb[:, t, :], bcA)
                    nc.vector.tensor_sub(out_sb[:, t, :], out_sb[:, t, :], bcC)

            def stage_load(b, stt):
                QT_sb = dbl.tile([P, DT, S], bf16, tag="qt", name="QT_sb")
                nc.sync.dma_start(QT_sb, QT[b])
                KT_sb = dbl.tile([P, DT, S], bf16, tag="kt", name="KT_sb")
                nc.sync.dma_start(KT_sb, KT[b])
                pT_sb = dbl.tile([4, S], bf16, tag="pt", name="pT_sb")
                nc.sync.dma_start(pT_sb, pT[b])
                stt.update(QT=QT_sb, KT=KT_sb, pT=pT_sb)

            def stage_proj(b, stt):
                QT_sb, KT_sb, pT_sb = stt["QT"], stt["KT"], stt["pT"]
                # natural Qh (for the attention residual)
                Qh = dbl.tile([P, DT, S], bf16, tag="qh", name="Qh")
                for t in range(DT):
                    ps = ps_mm.tile([P, S], f32, tag="mm", name="psq")
                    for kt in range(DT):
                        nc.tensor.matmul(
                            ps, Wq_sb[:, kt, t * P:(t + 1) * P],
                            QT_sb[:, kt, :],
                            start=(kt == 0), stop=(kt == DT - 1))
                    nc.vector.tensor_tensor(
                        Qh[:, t, :], ps,
                        bq_sb[:, t:t + 1].to_broadcast((P, S)), ALU.add)
                # aug tiles for scores: tile j partitions =
                # [Qh(2j) | Ph(2j) | Qh(2j+1) | Ph(2j+1)], biases folded via
                # the pT ones-row, so one K=64 matmul per (head, kt) yields
                # QK^T + PP^T in a single accumulation
                QA = dbl.tile([P, 4, S], bf16, tag="qa", name="QA")
                KA = dbl.tile([P, 4, S], bf16, tag="ka", name="KA")
                for j in range(4):
                    ps = ps_mm.tile([P, S], f32, tag="mm", name="psqa")
                    for kt in range(DT):
                        nc.tensor.matmul(ps, WqA_sb[:, kt, j, :],
                                         QT_sb[:, kt, :],
                                         start=(kt == 0), stop=False)
                    nc.tensor.matmul(ps, WpAq_sb[:, j, :], pT_sb,
                                     start=False, stop=True)
                    nc.vector.tensor_copy(QA[:, j, :], ps)
                    ps = ps_mm.tile([P, S], f32, tag="mm", name="pska")
                    for kt in range(DT):
                        nc.tensor.matmul(ps, WkA_sb[:, kt, j, :],
                                         KT_sb[:, kt, :],
                                         start=(kt == 0), stop=False)
                    nc.tensor.matmul(ps, WpAk_sb[:, j, :], pT_sb,
                                     start=False, stop=True)
                    nc.vector.tensor_copy(KA[:, j, :], ps)

                # V in natural layout [keys, feat], bf16, bias fused in move
                Vh = dbl.tile([P, ST, D], bf16, tag="vh", name="Vh")
                for st in range(ST):
                    ps = ps_mm.tile([P, S], f32, tag="mm", name="psv")
                    for kt in range(DT):
                        nc.tensor.matmul(
                            ps[:, :D], KT_sb[:, kt, st * P:(st + 1) * P],
                            Wv_sb[:, kt, :],
                            start=(kt == 0), stop=(kt == DT - 1))
                    nc.vector.tensor_add(Vh[:, st, :], ps[:, :D], bvb_sb)
                stt.update(Qh=Qh, QA=QA, KA=KA, Vh=Vh)

            def stage_attn(b, stt):
                Qh, QA, KA, Vh = stt["Qh"], stt["QA"], stt["KA"], stt["Vh"]
                OT = dbl.tile([P, DT, S], f32r, tag="ot", name="OT")
                for quad in range(2):
                    expS = [dbl.tile([P, ST, S], bf16, tag=f"e{i}",
                                     name=f"expS{i}") for i in range(4)]
                    den = ps_acc.tile([4, S], f32, tag="acc", name="den")
                    av = ps_av.tile([P, S], f32, tag="av", name="av")
                    sc_ps = {}
                    for kt in range(ST):
                        for h4 in range(4):
                            base = 64 * (h4 % 2)
                            j = 2 * quad + h4 // 2
                            ps = ps_mm.tile([P, S], f32, tag="mm", name="pssc")
                            sc_ps[h4] = ps
                            nc.tensor.matmul(
                                ps,
                                KA[base:base + 64, j, kt * P:(kt + 1) * P],
                                QA[base:base + 64, j, :],
                                start=True, stop=True,
                                tile_position=(base, 0))
                        for h4 in range(4):
                            nc.scalar.activation(expS[h4][:, kt, :],
                                                 sc_ps[h4], AF.Exp)
                        for h4 in range(4):
                            h = 4 * quad + h4
                            nc.tensor.matmul(
                                den, Ed4_sb[:, h4, :], expS[h4][:, kt, :],
                                start=(kt == 0 and h4 == 0),
                                stop=(kt == ST - 1 and h4 == 3),
                                skip_group_check=True)
                            nc.tensor.matmul(
                                av[32 * h4:32 * h4 + 32, :],
                                Vh[:, kt, 32 * h:32 * h + 32],
                                expS[h4][:, kt, :],
                                start=(kt == 0), stop=(kt == ST - 1),
                                tile_position=(0, 32 * h4),
                                skip_group_check=True)

                    # bc = broadcast(1/den): recip fused into the move
                    r4f = dbl.tile([4, S], f32, tag="r4f", name="r4f")
                    nc.vector.reciprocal_approx_fast(r4f, den[0:4, :])
                    r4 = dbl.tile([4, S], f32r, tag="r4", name="r4")
                    nc.vector.tensor_copy(r4, r4f)
                    bc = ps_bc.tile([P, S], f32, tag="bc", name="bc")
                    nc.tensor.matmul(bc, EB4_sb, r4, start=True, stop=True)
                    bcS = dbl.tile([P, S], f32, tag="bcs", name="bcS")
                    nc.vector.tensor_copy(bcS, bc)
                    nc.vector.tensor_mul(OT[:, quad, :], av, bcS)
                    nc.vector.tensor_add(OT[:, quad, :], OT[:, quad, :],
                                         Qh[:, quad, :])
                stt["OT"] = OT

            def stage_ffn(b, stt):
                OT = stt["OT"]
                LN1 = dbl.tile([P, DT, S], bf16, tag="ln1", name="LN1")
                layer_norm(OT, g0_sb, nb0_sb, LN1)
                # prefetch the gelu table; input dep on LN1's rstd pins this
                # after LN1's Exp in the ACT queue (scheduler can't hoist it)
                nc.scalar.activation(dummy, layer_norm.rstd[0:1, 0:1],
                                     AF.Gelu)

                G = dbl.tile([P, FT, S], bf16, tag="g", bufs=1, name="G")
                for ft in range(FT):
                    ps = ps_mm.tile([P, S], f32, tag="mm", name="psf")
                    for t in range(DT):
                        nc.tensor.matmul(
                            ps, W1_sb[:, t, ft * P:(ft + 1) * P],
                            LN1[:, t, :],
                            start=(t == 0), stop=(t == DT - 1))
                    nc.scalar.activation(G[:, ft, :], ps, AF.Gelu,
                                         bias=b1_sb[:, ft:ft + 1])
                # prefetch the ln/exp table; dep on the last gelu's output
                # pins it after the gelu loop in the ACT queue
                nc.scalar.activation(dummy, G[0:1, FT - 1, 0:1], AF.Ln)
                Z = dbl.tile([P, DT, S], f32r, tag="z", bufs=1, name="Z")
                for t in range(DT):
                    ps = ps_mm.tile([P, S], f32, tag="mm", name="psf2")
                    for ft in range(FT):
                        nc.tensor.matmul(
                            ps, W2_sb[:, ft, t * P:(t + 1) * P],
                            G[:, ft, :],
                            start=(ft == 0), stop=(ft == FT - 1))
                    nc.vector.tensor_add(Z[:, t, :], ps, LN1[:, t, :])
                    nc.vector.tensor_tensor(
                        Z[:, t, :], Z[:, t, :],
                        b2_sb[:, t:t + 1].to_broadcast((P, S)), ALU.add)
                stt["Z"] = Z

            def stage_out(b, stt):
                OUT = dbl.tile([P, DT, S], f32, tag="out", name="OUT")
                layer_norm(stt["Z"], g1_sb, nb1_sb, OUT)
                for t in range(DT):
                    nc.sync.dma_start(outT[b][:, t, :], OUT[:, t, :])

            # software pipeline: emit batch b+1's projections before
            # batch b's LN1 (fills the LN stats stall on the PE queue) and
            # batch b+1's attention before batch b's LN2
            sts = [dict() for _ in range(BL)]
            stage_load(0, sts[0])
            WqA_sb = load(WqA, (P, DT, 4, P))
            WkA_sb = load(WkA, (P, DT, 4, P))
            WpAq_sb = load(WpAq, (4, 4, P))
            WpAk_sb = load(WpAk, (4, 4, P))
            Wv_sb = load(Wv, (P, DT, D))
            bvb_sb = loadj(bvb, (P, D))
            Ed4_sb = load(Ed4, (P, 4, 4))
            EB4_sb = load(EB4, (4, P))
            one33_sb = load(one33, (P, 2, 33))
            onesS_sb = load(onesS, (1, S))
            g0_sb = load(g0r, (1, D))
            nb0_sb = load(nb0, (1, D))
            g1_sb = load(g1r, (1, D))
            nb1_sb = load(nb1, (1, D))
            stage_proj(0, sts[0])
            stage_attn(0, sts[0])
            W1_sb = load(W1, (P, DT, DFF))
            W2_sb = load(W2b, (P, FT, D))
            b1_sb = loadj(b1, (P, FT))
            b2_sb = loadj(b2, (P, DT))
            for b in range(BL):
                if b + 1 < BL:
                    stage_load(b + 1, sts[b + 1])
                    stage_proj(b + 1, sts[b + 1])
                    stage_attn(b + 1, sts[b + 1])
                stage_ffn(b, sts[b])
                stage_out(b, sts[b])

    nc.finalize()
    return nc


_NC = None


def kernel(Q, K, p, Wq, bq, Wk, bk, Wv, bv, Wp, bp, g0, beta0, W1, b1, W2, b2,
           g1, beta1):
    global _NC
    if _NC is None:
        _NC = build_nc()

    f = np.float32
    bf = ml_dtypes.bfloat16

    def feat_tiles(x):  # [B, S, D] -> [B, P, DT, S]
        x = np.asarray(x, f).transpose(0, 2, 1).reshape(-1, DT, P, S)
        return np.ascontiguousarray(x.transpose(0, 2, 1, 3))

    def pp(vec, n):  # [n*P] -> [P, n]
        return np.ascontiguousarray(np.asarray(vec, f).reshape(n, P).T)

    def wmat(w, n, m):  # [n*P, m] -> [P, n, m]
        w = np.asarray(w, f).reshape(n, P, m)
        return np.ascontiguousarray(w.transpose(1, 0, 2))

    QTf = feat_tiles(Q)
    KTf = feat_tiles(K)
    # p padded to 4 channels; row 3 = ones (carries the PE-proj bias).
    # PE projection pre-scaled by 1/4 so PhPh^T carries the 1/sqrt(DV)=1/16.
    pTf = np.zeros((B, 4, S), f)
    pTf[:, :3, :] = np.transpose(np.asarray(p, f), (0, 2, 1))
    pTf[:, 3, :] = 1.0
    # aug score weights: out tile j partitions =
    # [Qh(2j) | Ph(2j) | Qh(2j+1) | Ph(2j+1)]; pT row3==1 carries biases;
    # PE term pre-scaled by 1/4 each side so PhPh^T carries 1/sqrt(DV)=1/16
    Wq_f = np.asarray(Wq, f)
    Wk_f = np.asarray(Wk, f)
    Wp_f = np.asarray(Wp, f) * 0.25
    bq_f = np.asarray(bq, f)
    bk_f = np.asarray(bk, f)
    bp_f = np.asarray(bp, f) * 0.25

    def aug_w(W):  # [D, D] -> [P, DT, 4, P] lhsT tiles
        out = np.zeros((P, DT, 4, P), f)
        Wt = W.reshape(DT, P, D)  # [kt, row, out_feature]
        for j in range(4):
            for hh in range(2):
                h = 2 * j + hh
                out[:, :, j, 64 * hh:64 * hh + 32] = \
                    Wt[:, :, 32 * h:32 * h + 32].transpose(1, 0, 2)
        return out

    def aug_p(bias):  # [4, 4, P]: rows 0-2 Wp at P slots, row 3 biases
        out = np.zeros((4, 4, P), f)
        for j in range(4):
            for hh in range(2):
                h = 2 * j + hh
                out[:3, j, 64 * hh + 32:64 * hh + 64] = \
                    Wp_f[:, 32 * h:32 * h + 32]
                out[3, j, 64 * hh:64 * hh + 32] = bias[32 * h:32 * h + 32]
                out[3, j, 64 * hh + 32:64 * hh + 64] = \
                    bp_f[32 * h:32 * h + 32]
        return out

    # EB4: r4 row h4 -> out partitions 32*h4..32*h4+31
    EB4m = np.zeros((4, P), f)
    for h4 in range(4):
        EB4m[h4, 32 * h4:32 * h4 + 32] = 1.0
    # Ed4[:, h4, :]: all-ones col h4 (masked partition-sum lhsT)
    Ed4m = np.zeros((P, 4, 4), f)
    for h4 in range(4):
        Ed4m[:, h4, h4] = 1.0
    # LN partition-sum weights (1/D folded in): [:,0,:] puts sum(x)/D at
    # out partition 0, [:,1,:] puts sum(x^2)/D at out partition 32
    one33m = np.zeros((P, 2, 33), f)
    one33m[:, 0, 0] = 1.0 / D
    one33m[:, 1, 32] = 1.0 / D

    shared = {
        "Wq": wmat(Wq, DT, D).astype(bf),
        "Wv": wmat(Wv, DT, D).astype(bf),
        "WqA": aug_w(Wq_f).astype(bf), "WkA": aug_w(Wk_f).astype(bf),
        "WpAq": aug_p(bq_f).astype(bf), "WpAk": aug_p(bk_f).astype(bf),
        "W1": wmat(W1, DT, DFF).astype(bf),
        "W2b": wmat(W2, FT, D).astype(bf),
        "bq": pp(bq, DT),
        "bvb": np.ascontiguousarray(np.broadcast_to(np.asarray(bv, f), (P, D))),
        "b1": pp(b1, FT), "b2": pp(b2, DT),
        "g0r": np.asarray(g0, f).reshape(1, D),
        "nb0": -np.asarray(beta0, f).reshape(1, D),
        "g1r": np.asarray(g1, f).reshape(1, D),
        "nb1": -np.asarray(beta1, f).reshape(1, D),
        "one33": one33m,
        "Ed4": Ed4m.astype(bf), "EB4": EB4m,
        "onesS": np.ones((1, S), f),
    }
    in_maps = []
    for c in range(NCORES):
        m = dict(shared)
        m["QT"] = np.ascontiguousarray(QTf[c * BL:(c + 1) * BL]).astype(bf)
        m["KT"] = np.ascontiguousarray(KTf[c * BL:(c + 1) * BL]).astype(bf)
        m["pT"] = np.ascontiguousarray(pTf[c * BL:(c + 1) * BL]).astype(bf)
        in_maps.append(m)

    import os
    trace = bool(os.environ.get("BASS_TRACE"))
    res = run_bass_kernel_spmd(_NC, in_maps, core_ids=list(range(NCORES)),
                               trace=trace)
    kernel._LAST = res
    outs = [res.results[c]["outT"] for c in range(NCORES)]
    full = np.concatenate(outs, axis=0)  # [B, P, DT, S]
    full = full.transpose(0, 2, 1, 3).reshape(B, D, S)  # [B, D, S]
    return np.ascontiguousarray(full.transpose(0, 2, 1))


# revision 29
# speedup vs baseline: 1.4867x; 1.0312x over previous
"""Trainium2 Bass kernel for nn_MAB_2121713844542 (dense transformer block).

Data-parallel over batch B=32 across 8 cores (4 batches/core), activations
transposed [feature, seq] so every matmul contracts on partitions.

v2 layout/engine plan (vs baseline):
  - softmax denominators accumulate into one [4,S] PSUM tile per quad via
    masked-ones matmuls; 1/d = exp(-ln(d)) on the scalar engine (exp/ln
    tables stay resident) -- removes the 107us of DVE RECIPROCAL.
  - AV matmuls write natural head positions (tile_position col=32*h4) so
    the softmax divide + Qh residual are two full-width DVE ops per quad.
  - LayerNorm: 1/D folded into the ones-matmul weights, m2/ln/exp on ACT,
    g/beta folded into K<=1/2 broadcast matmuls, 2-op DVE tail per tile.
  - expS/Vh/G/W2 in bf16 (same PE rate, half the SBUF) which buys full
    cross-batch double buffering (bufs=2) to keep the PE fed.
"""

import functools

import numpy as np
import ml_dtypes

import concourse.bass as bass
import concourse.mybir as mybir
import concourse.tile as tile
from concourse import bacc
from concourse import hw_specs as _hw_specs
from concourse.bass_utils import run_bass_kernel_spmd

# The act-table chooser greedily picks the first table containing the needed
# function, so an Exp..Ln..Exp sequence ping-pongs between `exp_and_others`
# and `natural_log` (9 table loads per batch, ~1.5us each). Empty every table
# except the two we want so exp/ln/square/copy all resolve to
# `natural_log_exp_and_others` (ids keep their canonical positions).
_KEEP_TABLES = ("natural_log_exp_and_others", "gelu_and_others")
_orig_get_tables = _hw_specs.get_activation_tables


@functools.cache
def _patched_get_tables(arch):
    tabs = _orig_get_tables(arch)
    return {k: (v if k in _KEEP_TABLES else set()) for k, v in tabs.items()}


_hw_specs.get_activation_tables = _patched_get_tables
bacc.get_activation_tables = _patched_get_tables

B, S, D, H, DH, DFF = 32, 512, 256, 8, 32, 2048
NCORES = 8
BL = B // NCORES
P = 128
DT = D // P     # 2 feature tiles
FT = DFF // P   # 16 ffn tiles
ST = S // P     # 4 seq tiles
f32 = mybir.dt.float32
f32r = mybir.dt.float32r
bf16 = mybir.dt.bfloat16
AF = mybir.ActivationFunctionType
ALU = mybir.AluOpType
EPS = 1e-5


def build_nc():
    nc = bacc.Bacc("TRN2", target_bir_lowering=False, debug=False,
                   num_devices=NCORES)

    QT = nc.dram_tensor("QT", (BL, P, DT, S), bf16, kind="ExternalInput")
    KT = nc.dram_tensor("KT", (BL, P, DT, S), bf16, kind="ExternalInput")
    pT = nc.dram_tensor("pT", (BL, 4, S), bf16, kind="ExternalInput")
    Wq = nc.dram_tensor("Wq", (P, DT, D), bf16, kind="ExternalInput")
    Wv = nc.dram_tensor("Wv", (P, DT, D), bf16, kind="ExternalInput")
    WqA = nc.dram_tensor("WqA", (P, DT, 4, P), bf16, kind="ExternalInput")
    WkA = nc.dram_tensor("WkA", (P, DT, 4, P), bf16, kind="ExternalInput")
    WpAq = nc.dram_tensor("WpAq", (4, 4, P), bf16, kind="ExternalInput")
    WpAk = nc.dram_tensor("WpAk", (4, 4, P), bf16, kind="ExternalInput")
    W1 = nc.dram_tensor("W1", (P, DT, DFF), bf16, kind="ExternalInput")
    W2b = nc.dram_tensor("W2b", (P, FT, D), bf16, kind="ExternalInput")
    bq = nc.dram_tensor("bq", (P, DT), f32, kind="ExternalInput")
    bvb = nc.dram_tensor("bvb", (P, D), f32, kind="ExternalInput")
    b1 = nc.dram_tensor("b1", (P, FT), f32, kind="ExternalInput")
    b2 = nc.dram_tensor("b2", (P, DT), f32, kind="ExternalInput")
    g0r = nc.dram_tensor("g0r", (1, D), f32r, kind="ExternalInput")
    nb0 = nc.dram_tensor("nb0", (1, D), f32r, kind="ExternalInput")
    g1r = nc.dram_tensor("g1r", (1, D), f32r, kind="ExternalInput")
    nb1 = nc.dram_tensor("nb1", (1, D), f32r, kind="ExternalInput")
    one33 = nc.dram_tensor("one33", (P, 2, 33), f32r, kind="ExternalInput")
    Ed4 = nc.dram_tensor("Ed4", (P, 4, 4), bf16, kind="ExternalInput")
    EB4 = nc.dram_tensor("EB4", (4, P), f32r, kind="ExternalInput")
    onesS = nc.dram_tensor("onesS", (1, S), f32r, kind="ExternalInput")
    outT = nc.dram_tensor("outT", (BL, P, DT, S), f32, kind="ExternalOutput")

    with tile.TileContext(nc) as tc:
        with (
            tc.tile_pool(name="singles", bufs=1) as singles,
            tc.tile_pool(name="dbl", bufs=2) as dbl,
            tc.tile_pool(name="ps_mm", bufs=3, space="PSUM") as ps_mm,
            tc.tile_pool(name="ps_acc", bufs=1, space="PSUM") as ps_acc,
            tc.tile_pool(name="ps_av", bufs=2, space="PSUM") as ps_av,
            tc.tile_pool(name="ps_bc", bufs=2, space="PSUM") as ps_bc,
        ):
            def load(dram, shape):
                t = singles.tile(list(shape), dram.dtype, name="w_" + dram.name)
                nc.sync.dma_start(t, dram[tuple(slice(None) for _ in shape)])
                return t

            # order matters: only what batch 0's proj needs loads first;
            # the 2MB of FFN weights stream in during batch-0 attention
            Wq_sb = load(Wq, (P, DT, D))

            def loadj(dram, shape):
                # stage through DVE so TensorScalar-ish consumers get a
                # same-engine dep (few sync-wait slots on those structs)
                st = load(dram, shape)
                t = singles.tile(list(shape), f32, name="j_" + dram.name)
                nc.vector.tensor_copy(t, st)
                return t

            bq_sb = loadj(bq, (P, DT))

            eps1 = singles.tile([1, 1], f32)
            nc.vector.memset(eps1, EPS)
            neghalf = singles.tile([1, 1], f32)
            nc.vector.memset(neghalf, -0.5)
            dummy = singles.tile([1, 1], f32)
            nc.vector.memset(dummy, 1.0)

            def layer_norm(x_sb, grow, nbrow, out_sb, filler=None):
                """out = LN(x) * g + beta.  x_sb [P,DT,S] f32r."""
                x2 = dbl.tile([P, DT, S], f32r, tag="x2", bufs=1, name="x2")
                for t in range(DT):
                    nc.vector.tensor_mul(x2[:, t, :], x_sb[:, t, :],
                                         x_sb[:, t, :])
                # partition 0 <- mean, partition 32 <- E[x^2]
                acc = ps_acc.tile([33, S], f32, tag="acc", name="acc")
                for t in range(DT):
                    nc.tensor.matmul(acc, one33_sb[:, 0, :], x_sb[:, t, :],
                                     start=(t == 0), stop=False)
                for t in range(DT):
                    nc.tensor.matmul(acc, one33_sb[:, 1, :], x2[:, t, :],
                                     start=False, stop=(t == DT - 1))
                rstd = dbl.tile([1, S], f32r, tag="rstd", name="rstd")
                m2v = dbl.tile([1, S], f32r, tag="m2v", name="m2v")
                cst = dbl.tile([1, S], f32r, tag="cst", name="cst")
                nc.scalar.activation(m2v, acc[0:1, :], AF.Square)
                nc.vector.tensor_sub(m2v, acc[32:33, :], m2v)
                nc.scalar.activation(acc[32:33, :], m2v, AF.Ln, bias=eps1)
                # rstd = exp(-0.5*ln(var+eps))
                nc.scalar.activation(rstd, acc[32:33, :], AF.Exp,
                                     scale=neghalf)
                # C = mean * rstd
                nc.vector.tensor_mul(cst, acc[0:1, :], rstd)
                layer_norm.rstd = rstd
                # independent matmuls emitted here keep the PE fed while the
                # Square->sub->Ln->Exp stats chain resolves (in-order queue)
                if filler is not None:
                    filler()
                for t in range(DT):
                    bcA = ps_bc.tile([P, S], f32, tag="bc", name="bcA")
                    nc.tensor.matmul(bcA, grow[0:1, t * P:(t + 1) * P],
                                     rstd, start=True, stop=True)
                    bcC = ps_bc.tile([P, S], f32, tag="bc", name="bcC")
                    nc.tensor.matmul(bcC, grow[0:1, t * P:(t + 1) * P],
                                     cst, start=True, stop=False)
                    nc.tensor.matmul(bcC, nbrow[0:1, t * P:(t + 1) * P],
                                     onesS_sb, start=False, stop=True)
                    # out = x*(g*rstd) - (g*mean*rstd - beta)
                    nc.vector.tensor_mul(out_sb[:, t, :], x_sb[:, t, :], bcA)
                    nc.vector.tensor_sub(out_sb[:, t, :], out_sb[:, t, :], bcC)

            def stage_load(b, stt):
                QT_sb = dbl.tile([P, DT, S], bf16, tag="qt", name="QT_sb")
                nc.sync.dma_start(QT_sb, QT[b])
                KT_sb = dbl.tile([P, DT, S], bf16, tag="kt", name="KT_sb")
                nc.sync.dma_start(KT_sb, KT[b])
                pT_sb = dbl.tile([4, S], bf16, tag="pt", name="pT_sb")
                nc.sync.dma_start(pT_sb, pT[b])
                stt.update(QT=QT_sb, KT=KT_sb, pT=pT_sb)

            def stage_proj(b, stt):
                QT_sb, KT_sb, pT_sb = stt["QT"], stt["KT"], stt["pT"]
                # natural Qh (for the attention residual)
                Qh = dbl.tile([P, DT, S], bf16, tag="qh", name="Qh")
                for t in range(DT):
                    ps = ps_mm.tile([P, S], f32, tag="mm", name="psq")
                    for kt in range(DT):
                        nc.tensor.matmul(
                            ps, Wq_sb[:, kt, t * P:(t + 1) * P],
                            QT_sb[:, kt, :],
                            start=(kt == 0), stop=(kt == DT - 1))
                    nc.vector.tensor_tensor(
                        Qh[:, t, :], ps,
                        bq_sb[:, t:t + 1].to_broadcast((P, S)), ALU.add)
                # aug tiles for scores: tile j partitions =
                # [Qh(2j) | Ph(2j) | Qh(2j+1) | Ph(2j+1)], biases folded via
                # the pT ones-row, so one K=64 matmul per (head, kt) yields
                # QK^T + PP^T in a single accumulation
                QA = dbl.tile([P, 4, S], bf16, tag="qa", name="QA")
                KA = dbl.tile([P, 4, S], bf16, tag="ka", name="KA")
                for j in range(4):
                    ps = ps_mm.tile([P, S], f32, tag="mm", name="psqa")
                    for kt in range(DT):
                        nc.tensor.matmul(ps, WqA_sb[:, kt, j, :],
                                         QT_sb[:, kt, :],
                                         start=(kt == 0), stop=False)
                    nc.tensor.matmul(ps, WpAq_sb[:, j, :], pT_sb,
                                     start=False, stop=True)
                    nc.vector.tensor_copy(QA[:, j, :], ps)
                    ps = ps_mm.tile([P, S], f32, tag="mm", name="pska")
                    for kt in range(DT):
                        nc.tensor.matmul(ps, WkA_sb[:, kt, j, :],
                                         KT_sb[:, kt, :],
                                         start=(kt == 0), stop=False)
                    nc.tensor.matmul(ps, WpAk_sb[:, j, :], pT_sb,
                                     start=False, stop=True)
                    nc.vector.tensor_copy(KA[:, j, :], ps)

                # V in natural layout [keys, feat], bf16, bias fused in move
                Vh = dbl.tile([P, ST, D], bf16, tag="vh", name="Vh")
                for st in range(ST):
                    ps = ps_mm.tile([P, S], f32, tag="mm", name="psv")
                    for kt in range(DT):
                        nc.tensor.matmul(
                            ps[:, :D], KT_sb[:, kt, st * P:(st + 1) * P],
                            Wv_sb[:, kt, :],
                            start=(kt == 0), stop=(kt == DT - 1))
                    nc.vector.tensor_add(Vh[:, st, :], ps[:, :D], bvb_sb)
                stt.update(Qh=Qh, QA=QA, KA=KA, Vh=Vh)

            def stage_attn(b, stt):
                Qh, QA, KA, Vh = stt["Qh"], stt["QA"], stt["KA"], stt["Vh"]
                OT = dbl.tile([P, DT, S], f32r, tag="ot", name="OT")
                for quad in range(2):
                    expS = [dbl.tile([P, ST, S], bf16, tag=f"e{i}",
                                     name=f"expS{i}") for i in range(4)]
                    den = ps_acc.tile([4, S], f32, tag="acc", name="den")
                    av = ps_av.tile([P, S], f32, tag="av", name="av")
                    sc_ps = {}
                    for kt in range(ST):
                        for h4 in range(4):
                            base = 64 * (h4 % 2)
                            j = 2 * quad + h4 // 2
                            ps = ps_mm.tile([P, S], f32, tag="mm", name="pssc")
                            sc_ps[h4] = ps
                            nc.tensor.matmul(
                                ps,
                                KA[base:base + 64, j, kt * P:(kt + 1) * P],
                                QA[base:base + 64, j, :],
                                start=True, stop=True,
                                tile_position=(base, 0))
                        for h4 in range(4):
                            nc.scalar.activation(expS[h4][:, kt, :],
                                                 sc_ps[h4], AF.Exp)
                        for h4 in range(4):
                            h = 4 * quad + h4
                            nc.tensor.matmul(
                                den, Ed4_sb[:, h4, :], expS[h4][:, kt, :],
                                start=(kt == 0 and h4 == 0),
                                stop=(kt == ST - 1 and h4 == 3),
                                skip_group_check=True)
                            nc.tensor.matmul(
                                av[32 * h4:32 * h4 + 32, :],
                                Vh[:, kt, 32 * h:32 * h + 32],
                                expS[h4][:, kt, :],
                                start=(kt == 0), stop=(kt == ST - 1),
                                tile_position=(0, 32 * h4),
                                skip_group_check=True)

                    # bc = broadcast(1/den): recip fused into the move
                    r4f = dbl.tile([4, S], f32, tag="r4f", name="r4f")
                    nc.vector.reciprocal_approx_fast(r4f, den[0:4, :])
                    r4 = dbl.tile([4, S], f32r, tag="r4", name="r4")
                    nc.vector.tensor_copy(r4, r4f)
                    bc = ps_bc.tile([P, S], f32, tag="bc", name="bc")
                    nc.tensor.matmul(bc, EB4_sb, r4, start=True, stop=True)
                    bcS = dbl.tile([P, S], f32, tag="bcs", name="bcS")
                    nc.vector.tensor_copy(bcS, bc)
                    nc.vector.tensor_mul(OT[:, quad, :], av, bcS)
                    nc.vector.tensor_add(OT[:, quad, :], OT[:, quad, :],
                                         Qh[:, quad, :])
                stt["OT"] = OT

            def stage_ffn(b, stt, nxt):
                OT = stt["OT"]
                if nxt is not None:
                    stage_load(b + 1, nxt)
                LN1 = dbl.tile([P, DT, S], bf16, tag="ln1", name="LN1")
                filler = None
                if nxt is not None:
                    filler = lambda: stage_proj(b + 1, nxt)
                layer_norm(OT, g0_sb, nb0_sb, LN1, filler=filler)
                # prefetch the gelu table; input dep on LN1's rstd pins this
                # after LN1's Exp in the ACT queue (scheduler can't hoist it)
                nc.scalar.activation(dummy, layer_norm.rstd[0:1, 0:1],
                                     AF.Gelu)

                G = dbl.tile([P, FT, S], bf16, tag="g", bufs=1, name="G")
                for ft in range(FT):
                    ps = ps_mm.tile([P, S], f32, tag="mm", name="psf")
                    for t in range(DT):
                        nc.tensor.matmul(
                            ps, W1_sb[:, t, ft * P:(ft + 1) * P],
                            LN1[:, t, :],
                            start=(t == 0), stop=(t == DT - 1))
                    nc.scalar.activation(G[:, ft, :], ps, AF.Gelu,
                                         bias=b1_sb[:, ft:ft + 1])
                # prefetch the ln/exp table; dep on the last gelu's output
                # pins it after the gelu loop in the ACT queue
                nc.scalar.activation(dummy, G[0:1, FT - 1, 0:1], AF.Ln)
                Z = dbl.tile([P, DT, S], f32r, tag="z", bufs=1, name="Z")
                for t in range(DT):
                    ps = ps_mm.tile([P, S], f32, tag="mm", name="psf2")
                    for ft in range(FT):
                        nc.tensor.matmul(
                            ps, W2_sb[:, ft, t * P:(t + 1) * P],
                            G[:, ft, :],
                            start=(ft == 0), stop=(ft == FT - 1))
                    nc.vector.tensor_add(Z[:, t, :], ps, LN1[:, t, :])
                    nc.vector.tensor_tensor(
                        Z[:, t, :], Z[:, t, :],
                        b2_sb[:, t:t + 1].to_broadcast((P, S)), ALU.add)
                stt["Z"] = Z

            def stage_out(b, stt, nxt):
                OUT = dbl.tile([P, DT, S], f32, tag="out", name="OUT")
                filler = None
                if nxt is not None:
                    filler = lambda: stage_attn(b + 1, nxt)
                layer_norm(stt["Z"], g1_sb, nb1_sb, OUT, filler=filler)
                for t in range(DT):
                    nc.sync.dma_start(outT[b][:, t, :], OUT[:, t, :])

            # software pipeline: emit batch b+1's projections before
            # batch b's LN1 (fills the LN stats stall on the PE queue) and
            # batch b+1's attention before batch b's LN2
            sts = [dict() for _ in range(BL)]
            stage_load(0, sts[0])
            WqA_sb = load(WqA, (P, DT, 4, P))
            WkA_sb = load(WkA, (P, DT, 4, P))
            WpAq_sb = load(WpAq, (4, 4, P))
            WpAk_sb = load(WpAk, (4, 4, P))
            Wv_sb = load(Wv, (P, DT, D))
            bvb_sb = loadj(bvb, (P, D))
            Ed4_sb = load(Ed4, (P, 4, 4))
            EB4_sb = load(EB4, (4, P))
            one33_sb = load(one33, (P, 2, 33))
            onesS_sb = load(onesS, (1, S))
            g0_sb = load(g0r, (1, D))
            nb0_sb = load(nb0, (1, D))
            g1_sb = load(g1r, (1, D))
            nb1_sb = load(nb1, (1, D))
            stage_proj(0, sts[0])
            stage_attn(0, sts[0])
            W1_sb = load(W1, (P, DT, DFF))
            W2_sb = load(W2b, (P, FT, D))
            b1_sb = loadj(b1, (P, FT))
            b2_sb = loadj(b2, (P, DT))
            for b in range(BL):
                nxt = sts[b + 1] if b + 1 < BL else None
                stage_ffn(b, sts[b], nxt)
                stage_out(b, sts[b], nxt)

    nc.finalize()
    return nc


_NC = None


def kernel(Q, K, p, Wq, bq, Wk, bk, Wv, bv, Wp, bp, g0, beta0, W1, b1, W2, b2,
           g1, beta1):
    global _NC
    if _NC is None:
        _NC = build_nc()

    f = np.float32
    bf = ml_dtypes.bfloat16

    def feat_tiles(x):  # [B, S, D] -> [B, P, DT, S]
        x = np.asarray(x, f).transpose(0, 2, 1).reshape(-1, DT, P, S)
        return np.ascontiguousarray(x.transpose(0, 2, 1, 3))

    def pp(vec, n):  # [n*P] -> [P, n]
        return np.ascontiguousarray(np.asarray(vec, f).reshape(n, P).T)

    def wmat(w, n, m):  # [n*P, m] -> [P, n, m]
        w = np.asarray(w, f).reshape(n, P, m)
        return np.ascontiguousarray(w.transpose(1, 0, 2))

    QTf = feat_tiles(Q)
    KTf = feat_tiles(K)
    # p padded to 4 channels; row 3 = ones (carries the PE-proj bias).
    # PE projection pre-scaled by 1/4 so PhPh^T carries the 1/sqrt(DV)=1/16.
    pTf = np.zeros((B, 4, S), f)
    pTf[:, :3, :] = np.transpose(np.asarray(p, f), (0, 2, 1))
    pTf[:, 3, :] = 1.0
    # aug score weights: out tile j partitions =
    # [Qh(2j) | Ph(2j) | Qh(2j+1) | Ph(2j+1)]; pT row3==1 carries biases;
    # PE term pre-scaled by 1/4 each side so PhPh^T carries 1/sqrt(DV)=1/16
    Wq_f = np.asarray(Wq, f)
    Wk_f = np.asarray(Wk, f)
    Wp_f = np.asarray(Wp, f) * 0.25
    bq_f = np.asarray(bq, f)
    bk_f = np.asarray(bk, f)
    bp_f = np.asarray(bp, f) * 0.25

    def aug_w(W):  # [D, D] -> [P, DT, 4, P] lhsT tiles
        out = np.zeros((P, DT, 4, P), f)
        Wt = W.reshape(DT, P, D)  # [kt, row, out_feature]
        for j in range(4):
            for hh in range(2):
                h = 2 * j + hh
                out[:, :, j, 64 * hh:64 * hh + 32] = \
                    Wt[:, :, 32 * h:32 * h + 32].transpose(1, 0, 2)
        return out

    def aug_p(bias):  # [4, 4, P]: rows 0-2 Wp at P slots, row 3 biases
        out = np.zeros((4, 4, P), f)
        for j in range(4):
            for hh in range(2):
                h = 2 * j + hh
                out[:3, j, 64 * hh + 32:64 * hh + 64] = \
                    Wp_f[:, 32 * h:32 * h + 32]
                out[3, j, 64 * hh:64 * hh + 32] = bias[32 * h:32 * h + 32]
                out[3, j, 64 * hh + 32:64 * hh + 64] = \
                    bp_f[32 * h:32 * h + 32]
        return out

    # EB4: r4 row h4 -> out partitions 32*h4..32*h4+31
    EB4m = np.zeros((4, P), f)
    for h4 in range(4):
        EB4m[h4, 32 * h4:32 * h4 + 32] = 1.0
    # Ed4[:, h4, :]: all-ones col h4 (masked partition-sum lhsT)
    Ed4m = np.zeros((P, 4, 4), f)
    for h4 in range(4):
        Ed4m[:, h4, h4] = 1.0
    # LN partition-sum weights (1/D folded in): [:,0,:] puts sum(x)/D at
    # out partition 0, [:,1,:] puts sum(x^2)/D at out partition 32
    one33m = np.zeros((P, 2, 33), f)
    one33m[:, 0, 0] = 1.0 / D
    one33m[:, 1, 32] = 1.0 / D

    shared = {
        "Wq": wmat(Wq, DT, D).astype(bf),
        "Wv": wmat(Wv, DT, D).astype(bf),
        "WqA": aug_w(Wq_f).astype(bf), "WkA": aug_w(Wk_f).astype(bf),
        "WpAq": aug_p(bq_f).astype(bf), "WpAk": aug_p(bk_f).astype(bf),
        "W1": wmat(W1, DT, DFF).astype(bf),
        "W2b": wmat(W2, FT, D).astype(bf),
        "bq": pp(bq, DT),
        "bvb": np.ascontiguousarray(np.broadcast_to(np.asarray(bv, f), (P, D))),
        "b1": pp(b1, FT), "b2": pp(b2, DT),
        "g0r": np.asarray(g0, f).reshape(1, D),
        "nb0": -np.asarray(beta0, f).reshape(1, D),
        "g1r": np.asarray(g1, f).reshape(1, D),
        "nb1": -np.asarray(beta1, f).reshape(1, D),
        "one33": one33m,
        "Ed4": Ed4m.astype(bf), "EB4": EB4m,
        "onesS": np.ones((1, S), f),
    }
    in_maps = []
    for c in range(NCORES):
        m = dict(shared)
        m["QT"] = np.ascontiguousarray(QTf[c * BL:(c + 1) * BL]).astype(bf)
        m["KT"] = np.ascontiguousarray(KTf[c * BL:(c + 1) * BL]).astype(bf)
        m["pT"] = np.ascontiguousarray(pTf[c * BL:(c + 1) * BL]).astype(bf)
        in_maps.append(m)

    import os
    trace = bool(os.environ.get("BASS_TRACE"))
    res = run_bass_kernel_spmd(_NC, in_maps, core_ids=list(range(NCORES)),
                               trace=trace)
    kernel._LAST = res
    outs = [res.results[c]["outT"] for c in range(NCORES)]
    full = np.concatenate(outs, axis=0)  # [B, P, DT, S]
    full = full.transpose(0, 2, 1, 3).reshape(B, D, S)  # [B, D, S]
    return np.ascontiguousarray(full.transpose(0, 2, 1))


# revision 30
# speedup vs baseline: 1.5178x; 1.0209x over previous
"""Trainium2 Bass kernel for nn_MAB_2121713844542 (dense transformer block).

Data-parallel over batch B=32 across 8 cores (4 batches/core), activations
transposed [feature, seq] so every matmul contracts on partitions.

v2 layout/engine plan (vs baseline):
  - softmax denominators accumulate into one [4,S] PSUM tile per quad via
    masked-ones matmuls; 1/d = exp(-ln(d)) on the scalar engine (exp/ln
    tables stay resident) -- removes the 107us of DVE RECIPROCAL.
  - AV matmuls write natural head positions (tile_position col=32*h4) so
    the softmax divide + Qh residual are two full-width DVE ops per quad.
  - LayerNorm: 1/D folded into the ones-matmul weights, m2/ln/exp on ACT,
    g/beta folded into K<=1/2 broadcast matmuls, 2-op DVE tail per tile.
  - expS/Vh/G/W2 in bf16 (same PE rate, half the SBUF) which buys full
    cross-batch double buffering (bufs=2) to keep the PE fed.
"""

import functools

import numpy as np
import ml_dtypes

import concourse.bass as bass
import concourse.mybir as mybir
import concourse.tile as tile
from concourse import bacc
from concourse import hw_specs as _hw_specs
from concourse.bass_utils import run_bass_kernel_spmd

# The act-table chooser greedily picks the first table containing the needed
# function, so an Exp..Ln..Exp sequence ping-pongs between `exp_and_others`
# and `natural_log` (9 table loads per batch, ~1.5us each). Empty every table
# except the two we want so exp/ln/square/copy all resolve to
# `natural_log_exp_and_others` (ids keep their canonical positions).
_KEEP_TABLES = ("natural_log_exp_and_others", "gelu_and_others")
_orig_get_tables = _hw_specs.get_activation_tables


@functools.cache
def _patched_get_tables(arch):
    tabs = _orig_get_tables(arch)
    return {k: (v if k in _KEEP_TABLES else set()) for k, v in tabs.items()}


_hw_specs.get_activation_tables = _patched_get_tables
bacc.get_activation_tables = _patched_get_tables

B, S, D, H, DH, DFF = 32, 512, 256, 8, 32, 2048
NCORES = 8
BL = B // NCORES
P = 128
DT = D // P     # 2 feature tiles
FT = DFF // P   # 16 ffn tiles
ST = S // P     # 4 seq tiles
f32 = mybir.dt.float32
f32r = mybir.dt.float32r
bf16 = mybir.dt.bfloat16
AF = mybir.ActivationFunctionType
ALU = mybir.AluOpType
EPS = 1e-5


def build_nc(beta_zero):
    nc = bacc.Bacc("TRN2", target_bir_lowering=False, debug=False,
                   num_devices=NCORES)

    QT = nc.dram_tensor("QT", (BL, P, DT, S), bf16, kind="ExternalInput")
    KT = nc.dram_tensor("KT", (BL, P, DT, S), bf16, kind="ExternalInput")
    pT = nc.dram_tensor("pT", (BL, 4, S), bf16, kind="ExternalInput")
    Wq = nc.dram_tensor("Wq", (P, DT, D), bf16, kind="ExternalInput")
    Wv = nc.dram_tensor("Wv", (P, DT, D), bf16, kind="ExternalInput")
    WqA = nc.dram_tensor("WqA", (P, DT, 4, P), bf16, kind="ExternalInput")
    WkA = nc.dram_tensor("WkA", (P, DT, 4, P), bf16, kind="ExternalInput")
    WpAq = nc.dram_tensor("WpAq", (4, 4, P), bf16, kind="ExternalInput")
    WpAk = nc.dram_tensor("WpAk", (4, 4, P), bf16, kind="ExternalInput")
    W1 = nc.dram_tensor("W1", (P, DT, DFF), bf16, kind="ExternalInput")
    W2b = nc.dram_tensor("W2b", (P, FT, D), bf16, kind="ExternalInput")
    bq = nc.dram_tensor("bq", (P, DT), f32, kind="ExternalInput")
    bvb = nc.dram_tensor("bvb", (P, D), f32, kind="ExternalInput")
    b1 = nc.dram_tensor("b1", (P, FT), f32, kind="ExternalInput")
    b2 = nc.dram_tensor("b2", (P, DT), f32, kind="ExternalInput")
    g0r = nc.dram_tensor("g0r", (1, D), f32r, kind="ExternalInput")
    nb0 = nc.dram_tensor("nb0", (1, D), f32r, kind="ExternalInput")
    g1r = nc.dram_tensor("g1r", (1, D), f32r, kind="ExternalInput")
    nb1 = nc.dram_tensor("nb1", (1, D), f32r, kind="ExternalInput")
    one33 = nc.dram_tensor("one33", (P, 2, 33), f32r, kind="ExternalInput")
    Ed4 = nc.dram_tensor("Ed4", (P, 4, 4), bf16, kind="ExternalInput")
    EB4 = nc.dram_tensor("EB4", (4, P), f32r, kind="ExternalInput")
    onesS = nc.dram_tensor("onesS", (1, S), f32r, kind="ExternalInput")
    outT = nc.dram_tensor("outT", (BL, P, DT, S), f32, kind="ExternalOutput")

    with tile.TileContext(nc) as tc:
        with (
            tc.tile_pool(name="singles", bufs=1) as singles,
            tc.tile_pool(name="dbl", bufs=2) as dbl,
            tc.tile_pool(name="ps_mm", bufs=3, space="PSUM") as ps_mm,
            tc.tile_pool(name="ps_acc", bufs=1, space="PSUM") as ps_acc,
            tc.tile_pool(name="ps_av", bufs=2, space="PSUM") as ps_av,
            tc.tile_pool(name="ps_bc", bufs=2, space="PSUM") as ps_bc,
        ):
            def load(dram, shape):
                t = singles.tile(list(shape), dram.dtype, name="w_" + dram.name)
                nc.sync.dma_start(t, dram[tuple(slice(None) for _ in shape)])
                return t

            # order matters: only what batch 0's proj needs loads first;
            # the 2MB of FFN weights stream in during batch-0 attention
            Wq_sb = load(Wq, (P, DT, D))
            # (bq loadj emitted right below, before the batch-0 input DMAs)

            def loadj(dram, shape):
                # stage through DVE so TensorScalar-ish consumers get a
                # same-engine dep (few sync-wait slots on those structs)
                st = load(dram, shape)
                t = singles.tile(list(shape), f32, name="j_" + dram.name)
                nc.vector.tensor_copy(t, st)
                return t

            bq_sb = loadj(bq, (P, DT))

            eps1 = singles.tile([1, 1], f32)
            nc.vector.memset(eps1, EPS)
            neghalf = singles.tile([1, 1], f32)
            nc.vector.memset(neghalf, -0.5)
            dummy = singles.tile([1, 1], f32)
            nc.vector.memset(dummy, 1.0)

            def layer_norm(x_sb, grow, nbrow, out_sb, filler=None):
                """out = LN(x) * g + beta.  x_sb [P,DT,S] f32r."""
                x2 = dbl.tile([P, DT, S], f32r, tag="x2", bufs=1, name="x2")
                for t in range(DT):
                    nc.gpsimd.tensor_tensor(x2[:, t, :], x_sb[:, t, :],
                                            x_sb[:, t, :], ALU.mult)
                # partition 0 <- mean, partition 32 <- E[x^2]
                acc = ps_acc.tile([33, S], f32, tag="acc", name="acc")
                for t in range(DT):
                    nc.tensor.matmul(acc, one33_sb[:, 0, :], x_sb[:, t, :],
                                     start=(t == 0), stop=False)
                for t in range(DT):
                    nc.tensor.matmul(acc, one33_sb[:, 1, :], x2[:, t, :],
                                     start=False, stop=(t == DT - 1))
                rstd = dbl.tile([1, S], f32r, tag="rstd", name="rstd")
                m2v = dbl.tile([1, S], f32r, tag="m2v", name="m2v")
                cst = dbl.tile([1, S], f32r, tag="cst", name="cst")
                nc.scalar.activation(m2v, acc[0:1, :], AF.Square)
                nc.vector.tensor_sub(m2v, acc[32:33, :], m2v)
                nc.scalar.activation(acc[32:33, :], m2v, AF.Ln, bias=eps1)
                # rstd = exp(-0.5*ln(var+eps))
                nc.scalar.activation(rstd, acc[32:33, :], AF.Exp,
                                     scale=neghalf)
                # C = mean * rstd
                nc.vector.tensor_mul(cst, acc[0:1, :], rstd)
                layer_norm.rstd = rstd
                # independent matmuls emitted here keep the PE fed while the
                # Square->sub->Ln->Exp stats chain resolves (in-order queue)
                if filler is not None:
                    filler()
                for t in range(DT):
                    bcA = ps_bc.tile([P, S], f32, tag="bc", name="bcA")
                    nc.tensor.matmul(bcA, grow[0:1, t * P:(t + 1) * P],
                                     rstd, start=True, stop=True)
                    bcC = ps_bc.tile([P, S], f32, tag="bc", name="bcC")
                    nc.tensor.matmul(bcC, grow[0:1, t * P:(t + 1) * P],
                                     cst, start=True, stop=beta_zero)
                    if not beta_zero:
                        nc.tensor.matmul(bcC, nbrow[0:1, t * P:(t + 1) * P],
                                         onesS_sb, start=False, stop=True)
                    # out = x*(g*rstd) - (g*mean*rstd - beta)
                    nc.vector.tensor_mul(out_sb[:, t, :], x_sb[:, t, :], bcA)
                    nc.vector.tensor_sub(out_sb[:, t, :], out_sb[:, t, :], bcC)

            def stage_load(b, stt):
                QT_sb = dbl.tile([P, DT, S], bf16, tag="qt", name="QT_sb")
                nc.sync.dma_start(QT_sb, QT[b])
                KT_sb = dbl.tile([P, DT, S], bf16, tag="kt", name="KT_sb")
                nc.sync.dma_start(KT_sb, KT[b])
                pT_sb = dbl.tile([4, S], bf16, tag="pt", name="pT_sb")
                nc.sync.dma_start(pT_sb, pT[b])
                stt.update(QT=QT_sb, KT=KT_sb, pT=pT_sb)

            def stage_proj(b, stt):
                QT_sb, KT_sb, pT_sb = stt["QT"], stt["KT"], stt["pT"]
                # natural Qh (for the attention residual)
                Qh = dbl.tile([P, DT, S], bf16, tag="qh", name="Qh")
                for t in range(DT):
                    ps = ps_mm.tile([P, S], f32, tag="mm", name="psq")
                    for kt in range(DT):
                        nc.tensor.matmul(
                            ps, Wq_sb[:, kt, t * P:(t + 1) * P],
                            QT_sb[:, kt, :],
                            start=(kt == 0), stop=(kt == DT - 1))
                    nc.vector.tensor_tensor(
                        Qh[:, t, :], ps,
                        bq_sb[:, t:t + 1].to_broadcast((P, S)), ALU.add)
                # aug tiles for scores: tile j partitions =
                # [Qh(2j) | Ph(2j) | Qh(2j+1) | Ph(2j+1)], biases folded via
                # the pT ones-row, so one K=64 matmul per (head, kt) yields
                # QK^T + PP^T in a single accumulation
                QA = dbl.tile([P, 4, S], bf16, tag="qa", name="QA")
                KA = dbl.tile([P, 4, S], bf16, tag="ka", name="KA")
                for j in range(4):
                    ps = ps_mm.tile([P, S], f32, tag="mm", name="psqa")
                    for kt in range(DT):
                        nc.tensor.matmul(ps, WqA_sb[:, kt, j, :],
                                         QT_sb[:, kt, :],
                                         start=(kt == 0), stop=False)
                    nc.tensor.matmul(ps, WpAq_sb[:, j, :], pT_sb,
                                     start=False, stop=True)
                    nc.vector.tensor_copy(QA[:, j, :], ps)
                    ps = ps_mm.tile([P, S], f32, tag="mm", name="pska")
                    for kt in range(DT):
                        nc.tensor.matmul(ps, WkA_sb[:, kt, j, :],
                                         KT_sb[:, kt, :],
                                         start=(kt == 0), stop=False)
                    nc.tensor.matmul(ps, WpAk_sb[:, j, :], pT_sb,
                                     start=False, stop=True)
                    nc.vector.tensor_copy(KA[:, j, :], ps)

                # V in natural layout [keys, feat], bf16, bias fused in move
                Vh = dbl.tile([P, ST, D], bf16, tag="vh", name="Vh")
                for st in range(ST):
                    ps = ps_mm.tile([P, S], f32, tag="mm", name="psv")
                    for kt in range(DT):
                        nc.tensor.matmul(
                            ps[:, :D], KT_sb[:, kt, st * P:(st + 1) * P],
                            Wv_sb[:, kt, :],
                            start=(kt == 0), stop=(kt == DT - 1))
                    nc.vector.tensor_add(Vh[:, st, :], ps[:, :D], bvb_sb)
                stt.update(Qh=Qh, QA=QA, KA=KA, Vh=Vh)

            def stage_attn(b, stt):
                Qh, QA, KA, Vh = stt["Qh"], stt["QA"], stt["KA"], stt["Vh"]
                OT = dbl.tile([P, DT, S], f32r, tag="ot", name="OT")
                for quad in range(2):
                    expS = [dbl.tile([P, ST, S], bf16, tag=f"e{i}",
                                     name=f"expS{i}") for i in range(4)]
                    den = ps_acc.tile([4, S], f32, tag="acc", name="den")
                    av = ps_av.tile([P, S], f32, tag="av", name="av")
                    sc_ps = {}
                    for kt in range(ST):
                        for h4 in range(4):
                            base = 64 * (h4 % 2)
                            j = 2 * quad + h4 // 2
                            ps = ps_mm.tile([P, S], f32, tag="mm", name="pssc")
                            sc_ps[h4] = ps
                            nc.tensor.matmul(
                                ps,
                                KA[base:base + 64, j, kt * P:(kt + 1) * P],
                                QA[base:base + 64, j, :],
                                start=True, stop=True,
                                tile_position=(base, 0))
                        for h4 in range(4):
                            nc.scalar.activation(expS[h4][:, kt, :],
                                                 sc_ps[h4], AF.Exp)
                        for h4 in range(4):
                            h = 4 * quad + h4
                            nc.tensor.matmul(
                                den, Ed4_sb[:, h4, :], expS[h4][:, kt, :],
                                start=(kt == 0 and h4 == 0),
                                stop=(kt == ST - 1 and h4 == 3),
                                skip_group_check=True)
                            nc.tensor.matmul(
                                av[32 * h4:32 * h4 + 32, :],
                                Vh[:, kt, 32 * h:32 * h + 32],
                                expS[h4][:, kt, :],
                                start=(kt == 0), stop=(kt == ST - 1),
                                tile_position=(0, 32 * h4),
                                skip_group_check=True)

                    # bc = broadcast(1/den): recip fused into the move
                    r4f = dbl.tile([4, S], f32, tag="r4f", name="r4f")
                    nc.vector.reciprocal_approx_fast(r4f, den[0:4, :])
                    r4 = dbl.tile([4, S], f32r, tag="r4", name="r4")
                    nc.vector.tensor_copy(r4, r4f)
                    bc = ps_bc.tile([P, S], f32, tag="bc", name="bc")
                    nc.tensor.matmul(bc, EB4_sb, r4, start=True, stop=True)
                    bcS = dbl.tile([P, S], f32, tag="bcs", name="bcS")
                    nc.vector.tensor_copy(bcS, bc)
                    nc.vector.tensor_mul(OT[:, quad, :], av, bcS)
                    nc.vector.tensor_add(OT[:, quad, :], OT[:, quad, :],
                                         Qh[:, quad, :])
                stt["OT"] = OT

            def stage_ffn(b, stt, nxt):
                OT = stt["OT"]
                if nxt is not None:
                    stage_load(b + 1, nxt)
                LN1 = dbl.tile([P, DT, S], bf16, tag="ln1", name="LN1")
                filler = None
                if nxt is not None:
                    filler = lambda: stage_proj(b + 1, nxt)
                layer_norm(OT, g0_sb, nb0_sb, LN1, filler=filler)
                # prefetch the gelu table; input dep on LN1's rstd pins this
                # after LN1's Exp in the ACT queue (scheduler can't hoist it)
                nc.scalar.activation(dummy, layer_norm.rstd[0:1, 0:1],
                                     AF.Gelu)

                G = dbl.tile([P, FT, S], bf16, tag="g", bufs=1, name="G")
                for ft in range(FT):
                    ps = ps_mm.tile([P, S], f32, tag="mm", name="psf")
                    for t in range(DT):
                        nc.tensor.matmul(
                            ps, W1_sb[:, t, ft * P:(ft + 1) * P],
                            LN1[:, t, :],
                            start=(t == 0), stop=(t == DT - 1))
                    nc.scalar.activation(G[:, ft, :], ps, AF.Gelu,
                                         bias=b1_sb[:, ft:ft + 1])
                # prefetch the ln/exp table; dep on the last gelu's output
                # pins it after the gelu loop in the ACT queue
                nc.scalar.activation(dummy, G[0:1, FT - 1, 0:1], AF.Ln)
                Z = dbl.tile([P, DT, S], f32r, tag="z", bufs=1, name="Z")
                for t in range(DT):
                    ps = ps_mm.tile([P, S], f32, tag="mm", name="psf2")
                    for ft in range(FT):
                        nc.tensor.matmul(
                            ps, W2_sb[:, ft, t * P:(t + 1) * P],
                            G[:, ft, :],
                            start=(ft == 0), stop=(ft == FT - 1))
                    nc.vector.tensor_add(Z[:, t, :], ps, LN1[:, t, :])
                    nc.vector.tensor_tensor(
                        Z[:, t, :], Z[:, t, :],
                        b2_sb[:, t:t + 1].to_broadcast((P, S)), ALU.add)
                stt["Z"] = Z

            def stage_out(b, stt, nxt):
                OUT = dbl.tile([P, DT, S], f32, tag="out", name="OUT")
                filler = None
                if nxt is not None:
                    filler = lambda: stage_attn(b + 1, nxt)
                layer_norm(stt["Z"], g1_sb, nb1_sb, OUT, filler=filler)
                for t in range(DT):
                    nc.sync.dma_start(outT[b][:, t, :], OUT[:, t, :])

            # software pipeline: emit batch b+1's projections before
            # batch b's LN1 (fills the LN stats stall on the PE queue) and
            # batch b+1's attention before batch b's LN2
            sts = [dict() for _ in range(BL)]
            stage_load(0, sts[0])
            WqA_sb = load(WqA, (P, DT, 4, P))
            WkA_sb = load(WkA, (P, DT, 4, P))
            WpAq_sb = load(WpAq, (4, 4, P))
            WpAk_sb = load(WpAk, (4, 4, P))
            Wv_sb = load(Wv, (P, DT, D))
            bvb_sb = loadj(bvb, (P, D))
            Ed4_sb = load(Ed4, (P, 4, 4))
            EB4_sb = load(EB4, (4, P))
            one33_sb = load(one33, (P, 2, 33))
            onesS_sb = load(onesS, (1, S))
            g0_sb = load(g0r, (1, D))
            nb0_sb = load(nb0, (1, D))
            g1_sb = load(g1r, (1, D))
            nb1_sb = load(nb1, (1, D))
            stage_proj(0, sts[0])
            stage_attn(0, sts[0])
            W1_sb = load(W1, (P, DT, DFF))
            W2_sb = load(W2b, (P, FT, D))
            b1_sb = loadj(b1, (P, FT))
            b2_sb = loadj(b2, (P, DT))
            for b in range(BL):
                nxt = sts[b + 1] if b + 1 < BL else None
                stage_ffn(b, sts[b], nxt)
                stage_out(b, sts[b], nxt)

    nc.finalize()
    return nc


_NC = None


def kernel(Q, K, p, Wq, bq, Wk, bk, Wv, bv, Wp, bp, g0, beta0, W1, b1, W2, b2,
           g1, beta1):
    global _NC
    beta_zero = bool(np.all(np.asarray(beta0) == 0)
                     and np.all(np.asarray(beta1) == 0))
    if _NC is None:
        _NC = build_nc(beta_zero)

    f = np.float32
    bf = ml_dtypes.bfloat16

    def feat_tiles(x):  # [B, S, D] -> [B, P, DT, S]
        x = np.asarray(x, f).transpose(0, 2, 1).reshape(-1, DT, P, S)
        return np.ascontiguousarray(x.transpose(0, 2, 1, 3))

    def pp(vec, n):  # [n*P] -> [P, n]
        return np.ascontiguousarray(np.asarray(vec, f).reshape(n, P).T)

    def wmat(w, n, m):  # [n*P, m] -> [P, n, m]
        w = np.asarray(w, f).reshape(n, P, m)
        return np.ascontiguousarray(w.transpose(1, 0, 2))

    QTf = feat_tiles(Q)
    KTf = feat_tiles(K)
    # p padded to 4 channels; row 3 = ones (carries the PE-proj bias).
    # PE projection pre-scaled by 1/4 so PhPh^T carries the 1/sqrt(DV)=1/16.
    pTf = np.zeros((B, 4, S), f)
    pTf[:, :3, :] = np.transpose(np.asarray(p, f), (0, 2, 1))
    pTf[:, 3, :] = 1.0
    # aug score weights: out tile j partitions =
    # [Qh(2j) | Ph(2j) | Qh(2j+1) | Ph(2j+1)]; pT row3==1 carries biases;
    # PE term pre-scaled by 1/4 each side so PhPh^T carries 1/sqrt(DV)=1/16
    Wq_f = np.asarray(Wq, f)
    Wk_f = np.asarray(Wk, f)
    Wp_f = np.asarray(Wp, f) * 0.25
    bq_f = np.asarray(bq, f)
    bk_f = np.asarray(bk, f)
    bp_f = np.asarray(bp, f) * 0.25

    def aug_w(W):  # [D, D] -> [P, DT, 4, P] lhsT tiles
        out = np.zeros((P, DT, 4, P), f)
        Wt = W.reshape(DT, P, D)  # [kt, row, out_feature]
        for j in range(4):
            for hh in range(2):
                h = 2 * j + hh
                out[:, :, j, 64 * hh:64 * hh + 32] = \
                    Wt[:, :, 32 * h:32 * h + 32].transpose(1, 0, 2)
        return out

    def aug_p(bias):  # [4, 4, P]: rows 0-2 Wp at P slots, row 3 biases
        out = np.zeros((4, 4, P), f)
        for j in range(4):
            for hh in range(2):
                h = 2 * j + hh
                out[:3, j, 64 * hh + 32:64 * hh + 64] = \
                    Wp_f[:, 32 * h:32 * h + 32]
                out[3, j, 64 * hh:64 * hh + 32] = bias[32 * h:32 * h + 32]
                out[3, j, 64 * hh + 32:64 * hh + 64] = \
                    bp_f[32 * h:32 * h + 32]
        return out

    # EB4: r4 row h4 -> out partitions 32*h4..32*h4+31
    EB4m = np.zeros((4, P), f)
    for h4 in range(4):
        EB4m[h4, 32 * h4:32 * h4 + 32] = 1.0
    # Ed4[:, h4, :]: all-ones col h4 (masked partition-sum lhsT)
    Ed4m = np.zeros((P, 4, 4), f)
    for h4 in range(4):
        Ed4m[:, h4, h4] = 1.0
    # LN partition-sum weights (1/D folded in): [:,0,:] puts sum(x)/D at
    # out partition 0, [:,1,:] puts sum(x^2)/D at out partition 32
    one33m = np.zeros((P, 2, 33), f)
    one33m[:, 0, 0] = 1.0 / D
    one33m[:, 1, 32] = 1.0 / D

    shared = {
        "Wq": wmat(Wq, DT, D).astype(bf),
        "Wv": wmat(Wv, DT, D).astype(bf),
        "WqA": aug_w(Wq_f).astype(bf), "WkA": aug_w(Wk_f).astype(bf),
        "WpAq": aug_p(bq_f).astype(bf), "WpAk": aug_p(bk_f).astype(bf),
        "W1": wmat(W1, DT, DFF).astype(bf),
        "W2b": wmat(W2, FT, D).astype(bf),
        "bq": pp(bq, DT),
        "bvb": np.ascontiguousarray(np.broadcast_to(np.asarray(bv, f), (P, D))),
        "b1": pp(b1, FT), "b2": pp(b2, DT),
        "g0r": np.asarray(g0, f).reshape(1, D),
        "nb0": -np.asarray(beta0, f).reshape(1, D),
        "g1r": np.asarray(g1, f).reshape(1, D),
        "nb1": -np.asarray(beta1, f).reshape(1, D),
        "one33": one33m,
        "Ed4": Ed4m.astype(bf), "EB4": EB4m,
        "onesS": np.ones((1, S), f),
    }
    in_maps = []
    for c in range(NCORES):
        m = dict(shared)
        m["QT"] = np.ascontiguousarray(QTf[c * BL:(c + 1) * BL]).astype(bf)
        m["KT"] = np.ascontiguousarray(KTf[c * BL:(c + 1) * BL]).astype(bf)
        m["pT"] = np.ascontiguousarray(pTf[c * BL:(c + 1) * BL]).astype(bf)
        in_maps.append(m)

    import os
    trace = bool(os.environ.get("BASS_TRACE"))
    res = run_bass_kernel_spmd(_NC, in_maps, core_ids=list(range(NCORES)),
                               trace=trace)
    kernel._LAST = res
    outs = [res.results[c]["outT"] for c in range(NCORES)]
    full = np.concatenate(outs, axis=0)  # [B, P, DT, S]
    full = full.transpose(0, 2, 1, 3).reshape(B, D, S)  # [B, D, S]
    return np.ascontiguousarray(full.transpose(0, 2, 1))
